# revision 1
# baseline (speedup 1.0000x reference)
"""GCNNet forward on 8 Trainium2 NeuronCores (Bass/Tile).

Sharding: nodes in 8 contiguous blocks (SHARD rows each, tail zero-padded);
edges assigned to the core owning their *destination*. Per conv layer:

  hw_pre = dinv * (BN(h) @ W)   -- BN folded into the weights (W' = diag(s)W,
                                   c = t@W); per-feature stats via PE
                                   ones-matmul partition reduction + a tiny
                                   AllReduce of [feat, 2] sums
  AllGather hw_pre -> hw_full   -- gather source, tile-layout rows
  per-edge messages come in via dma_gather (SWDGE custom ucode, int16
  indices, sources bucketed into 32768-row chunks)
  segment-sum on the PE: per 128-edge tile a one-hot matrix (DVE
  iota-compare against the dst slot) is matmul'd into a PSUM accumulator
  per (chunk, dst-block) group, then spilled into an SBUF accumulator that
  was seeded with the self-loop term (hw_pre itself)
  h_next = relu(dinv * agg + b) * valid

The GCN normalization is separable (norm_e = dinv[src] * dinv[dst] for kept
edges), so no per-edge float arithmetic runs on any compute engine — only
the gather DMA and the PE segment-sum. Pooling reuses the one-hot matmul
over batch ids + an AllReduce; the tiny 2-layer head runs redundantly on
every core.

All plain DMAs use nc.gpsimd (SWDGE): HWDGE (nc.sync) DMAs alongside the
custom SWDGE gather ucode crash the device (empirically bisected).
"""
import sys

sys.path.insert(0, "/opt/trn_rl_repo")

import ml_dtypes
import numpy as np

import concourse.bacc as bacc
import concourse.mybir as mybir
import concourse.tile as tile

F32 = mybir.dt.float32
BF16 = mybir.dt.bfloat16
I16 = mybir.dt.int16

NCORES = 8
CHUNK = 32768          # gather-index range per int16 chunk
import os
CALL_TILES = int(os.environ.get("CALL_TILES", "64"))  # tiles per dma_gather call
MSG_BUFS = int(os.environ.get("MSG_BUFS", "4"))
SINGLE_PACKET = os.environ.get("SINGLE_PACKET", "0") == "1"
HWDGE = os.environ.get("HWDGE", "0") == "1"  # plain DMAs on SP HWDGE
OH_BATCH = int(os.environ.get("OHB", "8"))  # tiles per DVE one-hot op
PAD_SLOT = 200         # one-hot slot for padding edges (matches nothing)
EPS = 1e-5
SCRATCH = 32768        # SWDGE descriptor carveout bytes/partition
REPEAT_MP = 1          # timing: repeat the message-passing phase per layer
REPEAT_ALL = 1         # timing: repeat the whole forward pass in one program
NQUEUES = int(os.environ.get("NQUEUES", "4"))  # SWDGE queues
ABLATE = ""            # timing: comma-list of no_gather|gather_only|no_ag|no_stats_ar|no_mp


def _abl(flag):
    return flag in ABLATE.split(",")


def _wrap_idx(a):
    """int16 indices -> SWDGE layout [128, n/16] (16-wrapped, 8x replicated)."""
    assert a.size % 16 == 0
    w = a.reshape(-1, 16).T.copy()
    return np.ascontiguousarray(np.tile(w, (8, 1)))


def _tab128(a, nt):
    """[nt*128] -> [128, nt] tile-column table (node l -> [l%128, l//128])."""
    return np.ascontiguousarray(a.reshape(nt, 128).T)


def preprocess(inputs):
    x = np.asarray(inputs["x"], np.float32)
    ei = np.asarray(inputs["edge_index"], np.int64)
    batch = np.asarray(inputs["batch"], np.int64)
    N, F = x.shape
    W_conv = np.asarray(inputs["W_conv"], np.float32)
    H = W_conv.shape[-1]
    W_cls = np.asarray(inputs["W_cls"], np.float32)
    C = W_cls.shape[-1]
    G = int(np.asarray(inputs["num_graphs"]))
    assert G <= 128 and F <= 128 and H <= 128

    SHARD = -(-N // (NCORES * 128)) * 128
    NT = SHARD // 128
    NPAD = NCORES * SHARD
    NCHUNK = -(-NPAD // CHUNK)

    row, col = ei[0], ei[1]
    keep = row != col
    row = row[keep]
    col = col[keep]

    deg = (np.bincount(row, minlength=N) + 1).astype(np.float32)
    dinv = (np.float32(1.0) / np.sqrt(deg)).astype(np.float32)
    dinv_pad = np.zeros(NPAD, np.float32)
    valid_pad = np.zeros(NPAD, np.float32)
    batch_pad = np.full(NPAD, PAD_SLOT, np.int16)
    dinv_pad[:N] = dinv
    valid_pad[:N] = 1.0
    batch_pad[:N] = batch.astype(np.int16)

    # hw_full rows use tile-layout: node l = t*128 + p on core k sits at
    # global row k*SHARD + p*NT + t.
    nglob = np.arange(NPAD, dtype=np.int64)
    n_local = nglob % SHARD
    tl_row = (nglob // SHARD) * SHARD + (n_local % 128) * NT + n_local // 128

    src_row = tl_row[row]
    dst_core = col // SHARD
    dst_local = col % SHARD

    NBLK = NT
    per_core = []
    cnts = np.zeros((NCORES, NCHUNK * NBLK), np.int64)
    for k in range(NCORES):
        m = dst_core == k
        r = src_row[m]
        c = dst_local[m]
        key = (r // CHUNK) * NBLK + (c >> 7)
        order = np.argsort(key, kind="stable")
        per_core.append((r[order], c[order], key[order]))
        cnts[k] = np.bincount(key, minlength=NCHUNK * NBLK)

    # Tightly-packed shared schedule: per (chunk, dst-block) group, slots =
    # max count across cores (no per-group 128-rounding); groups packed
    # back-to-back within each chunk, chunk tail padded to a tile boundary.
    # A 128-edge tile may span several groups; the one-hot matmul runs per
    # (tile, group-segment) on the partition subrange.
    gs_max = cnts.max(axis=0)

    calls = []           # (chunk, tile_off, n_tiles)
    seg_meta = []        # (gt, p0, p1, first, last, blk) per segment
    goff = np.zeros(NCHUNK * NBLK, np.int64)   # global slot offset per group
    n_tiles = 0
    for ch in range(NCHUNK):
        ch_slot0 = n_tiles * 128
        off = 0
        for blk in range(NBLK):
            g = ch * NBLK + blk
            goff[g] = ch_slot0 + off
            off += int(gs_max[g])
        S_ch = off
        nt_ch = -(-S_ch // 128)
        for blk in range(NBLK):
            g = ch * NBLK + blk
            s = int(gs_max[g])
            if s == 0:
                continue
            s0 = int(goff[g]) - ch_slot0
            s1 = s0 + s
            t0, t1 = s0 // 128, (s1 - 1) // 128
            for t in range(t0, t1 + 1):
                p0 = max(0, s0 - t * 128)
                p1 = min(128, s1 - t * 128)
                seg_meta.append(
                    (n_tiles + t, p0, p1, t == t0, t == t1, blk))
        off2 = 0
        while off2 < nt_ch:
            n = min(CALL_TILES, nt_ch - off2)
            calls.append((ch, n_tiles + off2, n))
            off2 += n
        n_tiles += nt_ch
    TOT = n_tiles * 128
    seg_meta.sort(key=lambda s: (s[0], s[1]))
    n_segs = len(seg_meta)

    # per-call segment lists: (seg_col0, [(tloc, first, last, blk), ...])
    call_segs = []
    si = 0
    for (ch, tile_off, ntl) in calls:
        s0 = si
        segs = []
        while si < n_segs and seg_meta[si][0] < tile_off + ntl:
            gt, p0, p1, first, last, blk = seg_meta[si]
            segs.append((gt - tile_off, first, last, blk))
            si += 1
        call_segs.append((s0, segs))
    assert si == n_segs

    src_tab = []
    slot_tab = []
    for k in range(NCORES):
        r, c, key = per_core[k]
        src_s = np.zeros(TOT, np.int64)          # pads gather row 0 of chunk
        slot_s = np.full(TOT, PAD_SLOT, np.int16)
        kcnt = cnts[k]
        start_of_group = np.concatenate([[0], np.cumsum(kcnt)[:-1]])
        within = np.arange(r.size, dtype=np.int64) - np.repeat(start_of_group, kcnt)
        pos = goff[key] + within
        src_s[pos] = r % CHUNK
        slot_s[pos] = (c & 127).astype(np.int16)
        src_tab.append(_wrap_idx(src_s.astype(np.int16)))
        seg_slot = np.full((n_segs, 128), PAD_SLOT, np.int16)
        for s, (gt, p0, p1, _f, _l, _b) in enumerate(seg_meta):
            seg_slot[s, p0:p1] = slot_s[gt * 128 + p0: gt * 128 + p1]
        slot_tab.append(np.ascontiguousarray(seg_slot.T))

    meta = dict(
        N=N, F=F, H=H, C=C, G=G, SHARD=SHARD, NT=NT, NPAD=NPAD,
        NCHUNK=NCHUNK, n_tiles=n_tiles, n_segs=n_segs,
        call_segs=call_segs, calls=calls,
    )

    params = dict(
        W_feat=np.ascontiguousarray(np.asarray(inputs["W_feat"], np.float32)),
        W_conv_cat=np.ascontiguousarray(
            W_conv.transpose(1, 0, 2).reshape(H, 3 * H)),
        W_fc=np.ascontiguousarray(np.asarray(inputs["W_fc"], np.float32)),
        W_cls=np.ascontiguousarray(W_cls),
        b_conv_rep=np.ascontiguousarray(np.broadcast_to(
            np.asarray(inputs["b_conv"], np.float32)[None, :, :], (128, 3, H))),
        g_conv=np.ascontiguousarray(np.asarray(inputs["bn_conv_g"], np.float32).T),
        b2_conv=np.ascontiguousarray(np.asarray(inputs["bn_conv_b"], np.float32).T),
        g_feat=np.asarray(inputs["bn_feat_g"], np.float32).reshape(F, 1).copy(),
        b2_feat=np.asarray(inputs["bn_feat_b"], np.float32).reshape(F, 1).copy(),
        g_fc=np.asarray(inputs["bn_fc_g"], np.float32).reshape(H, 1).copy(),
        b2_fc=np.asarray(inputs["bn_fc_b"], np.float32).reshape(H, 1).copy(),
        g_hid=np.asarray(inputs["bn_hidden_g"], np.float32).reshape(H, 1).copy(),
        b2_hid=np.asarray(inputs["bn_hidden_b"], np.float32).reshape(H, 1).copy(),
        b_fc=np.asarray(inputs["b_fc"], np.float32).reshape(1, H).copy(),
        b_cls=np.asarray(inputs["b_cls"], np.float32).reshape(1, C).copy(),
        identity=np.eye(128, dtype=np.float32),
        iota=np.ascontiguousarray(
            np.broadcast_to(np.arange(128, dtype=np.int16)[None, :], (128, 128))),
        ones_col=np.ones((128, 1), np.float32),
        ones_row=np.ones((1, 128), np.float32),
        gvalid=(np.arange(128) < G).astype(np.float32).reshape(128, 1),
        eps_col=np.full((128, 1), EPS, np.float32),
    )

    x_pad = np.zeros((NPAD, F), np.float32)
    x_pad[:N] = x
    in_maps = []
    for k in range(NCORES):
        sl = slice(k * SHARD, (k + 1) * SHARD)
        m = dict(params)
        m["x"] = np.ascontiguousarray(x_pad[sl].astype(ml_dtypes.bfloat16))
        m["dinv_tab"] = _tab128(dinv_pad[sl], NT)
        m["valid_tab"] = _tab128(valid_pad[sl], NT)
        m["batch_tab"] = np.ascontiguousarray(batch_pad[sl].reshape(NT, 128).T)
        m["src_idx"] = src_tab[k]
        m["slot_tab"] = slot_tab[k]
        in_maps.append(m)

    return meta, in_maps


def build_program(meta):
    N, F, H, C, G = meta["N"], meta["F"], meta["H"], meta["C"], meta["G"]
    SHARD, NT, NPAD = meta["SHARD"], meta["NT"], meta["NPAD"]
    n_tiles = meta["n_tiles"]
    call_segs = meta["call_segs"]
    calls = meta["calls"]
    n_segs = meta["n_segs"]
    TOT = n_tiles * 128

    nc = bacc.Bacc("TRN2", target_bir_lowering=False, debug=False,
                   num_devices=NCORES, dynamic_dma_scratch_size=SCRATCH,
                   num_swdge_queues=NQUEUES)

    x_d = nc.dram_tensor("x", [SHARD, F], BF16, kind="ExternalInput")
    src_idx_d = nc.dram_tensor("src_idx", [128, TOT // 16], I16, kind="ExternalInput")
    slot_tab_d = nc.dram_tensor("slot_tab", [128, n_segs], I16, kind="ExternalInput")
    batch_tab_d = nc.dram_tensor("batch_tab", [128, NT], I16, kind="ExternalInput")
    dinv_tab_d = nc.dram_tensor("dinv_tab", [128, NT], F32, kind="ExternalInput")
    valid_tab_d = nc.dram_tensor("valid_tab", [128, NT], F32, kind="ExternalInput")
    Wf_d = nc.dram_tensor("W_feat", [F, H], F32, kind="ExternalInput")
    Wc_d = nc.dram_tensor("W_conv_cat", [H, 3 * H], F32, kind="ExternalInput")
    Wfc_d = nc.dram_tensor("W_fc", [H, H], F32, kind="ExternalInput")
    Wcls_d = nc.dram_tensor("W_cls", [H, C], F32, kind="ExternalInput")
    bconv_d = nc.dram_tensor("b_conv_rep", [128, 3, H], F32, kind="ExternalInput")
    gconv_d = nc.dram_tensor("g_conv", [H, 3], F32, kind="ExternalInput")
    b2conv_d = nc.dram_tensor("b2_conv", [H, 3], F32, kind="ExternalInput")
    gfeat_d = nc.dram_tensor("g_feat", [F, 1], F32, kind="ExternalInput")
    b2feat_d = nc.dram_tensor("b2_feat", [F, 1], F32, kind="ExternalInput")
    gfc_d = nc.dram_tensor("g_fc", [H, 1], F32, kind="ExternalInput")
    b2fc_d = nc.dram_tensor("b2_fc", [H, 1], F32, kind="ExternalInput")
    ghid_d = nc.dram_tensor("g_hid", [H, 1], F32, kind="ExternalInput")
    b2hid_d = nc.dram_tensor("b2_hid", [H, 1], F32, kind="ExternalInput")
    bfc_d = nc.dram_tensor("b_fc", [1, H], F32, kind="ExternalInput")
    bcls_d = nc.dram_tensor("b_cls", [1, C], F32, kind="ExternalInput")
    ident_d = nc.dram_tensor("identity", [128, 128], F32, kind="ExternalInput")
    iota_d = nc.dram_tensor("iota", [128, 128], I16, kind="ExternalInput")
    onesc_d = nc.dram_tensor("ones_col", [128, 1], F32, kind="ExternalInput")
    onesr_d = nc.dram_tensor("ones_row", [1, 128], F32, kind="ExternalInput")
    gvalid_d = nc.dram_tensor("gvalid", [128, 1], F32, kind="ExternalInput")
    eps_d = nc.dram_tensor("eps_col", [128, 1], F32, kind="ExternalInput")
    out_d = nc.dram_tensor("out", [128, C], F32, kind="ExternalOutput")

    hwpre_d = nc.dram_tensor("hwpre_dram", [SHARD, H], F32, kind="Internal")
    hw_full = nc.dram_tensor("hw_full", [NPAD, H], F32, kind="Internal",
                             addr_space="Shared")
    statF_l = nc.dram_tensor("statF_l", [F, 2], F32, kind="Internal")
    statF_s = nc.dram_tensor("statF_s", [F, 2], F32, kind="Internal",
                             addr_space="Shared")
    statH_l = [nc.dram_tensor(f"statH_l{i}", [H, 2], F32, kind="Internal")
               for i in range(3)]
    statH_s = [nc.dram_tensor(f"statH_s{i}", [H, 2], F32, kind="Internal",
                              addr_space="Shared") for i in range(3)]
    hgp_d = nc.dram_tensor("hgp_dram", [128, H], F32, kind="Internal")
    hg_sh = nc.dram_tensor("hg_sh", [128, H], F32, kind="Internal",
                           addr_space="Shared")

    dmae = nc.sync if HWDGE else nc.gpsimd
    RG = [list(range(NCORES))]
    AF = mybir.ActivationFunctionType
    ALU = mybir.AluOpType
    inv_n = 1.0 / float(N)
    inv_g = 1.0 / float(G)

    with tile.TileContext(nc) as tc:
        with tc.tile_pool(name="per", bufs=1) as per, \
             tc.tile_pool(name="st", bufs=2) as st:
            slot_sb = per.tile([128, n_segs], I16, tag="slots")
            idx_sb = per.tile([128, n_tiles * 8], I16, tag="idxall")
            dinv_sb = per.tile([128, NT], F32, tag="dinv")
            valid_sb = per.tile([128, NT], F32, tag="validt")
            batch_sb = per.tile([128, NT], I16, tag="batcht")
            iota_sb = per.tile([128, 128], I16, tag="iota")
            ident_sb = per.tile([128, 128], F32, tag="ident")
            onesc_sb = per.tile([128, 1], F32, tag="onesc")
            onesr_sb = per.tile([1, 128], F32, tag="onesr")
            gvalid_sb = per.tile([128, 1], F32, tag="gvalid")
            eps_sb = per.tile([128, 1], F32, tag="epsc")
            Wf_sb = per.tile([F, H], F32, tag="wf")
            Wc_sb = per.tile([H, 3 * H], F32, tag="wc")
            Wfc_sb = per.tile([H, H], F32, tag="wfc")
            Wcls_sb = per.tile([H, C], F32, tag="wcls")
            bconv_sb = per.tile([128, 3, H], F32, tag="bconv")
            gconv_sb = per.tile([H, 3], F32, tag="gconv")
            b2conv_sb = per.tile([H, 3], F32, tag="b2conv")
            gfeat_sb = per.tile([F, 1], F32, tag="gfeat")
            b2feat_sb = per.tile([F, 1], F32, tag="b2feat")
            gfc_sb = per.tile([H, 1], F32, tag="gfc")
            b2fc_sb = per.tile([H, 1], F32, tag="b2fc")
            ghid_sb = per.tile([H, 1], F32, tag="ghid")
            b2hid_sb = per.tile([H, 1], F32, tag="b2hid")
            bfc_sb = per.tile([1, H], F32, tag="bfc")
            bcls_sb = per.tile([1, C], F32, tag="bcls")

            for sb, d in [(slot_sb, slot_tab_d), (idx_sb, src_idx_d),
                          (dinv_sb, dinv_tab_d),
                          (valid_sb, valid_tab_d), (batch_sb, batch_tab_d),
                          (iota_sb, iota_d), (ident_sb, ident_d),
                          (onesc_sb, onesc_d), (onesr_sb, onesr_d),
                          (gvalid_sb, gvalid_d), (eps_sb, eps_d),
                          (Wf_sb, Wf_d), (Wc_sb, Wc_d), (Wfc_sb, Wfc_d),
                          (Wcls_sb, Wcls_d), (bconv_sb, bconv_d),
                          (gconv_sb, gconv_d), (b2conv_sb, b2conv_d),
                          (gfeat_sb, gfeat_d), (b2feat_sb, b2feat_d),
                          (gfc_sb, gfc_d), (b2fc_sb, b2fc_d),
                          (ghid_sb, ghid_d), (b2hid_sb, b2hid_d),
                          (bfc_sb, bfc_d), (bcls_sb, bcls_d)]:
                dmae.dma_start(sb[:], d[:])

            # ------------- helpers -------------
            def stats_reduce(get_tile, nt_count, Win, dst_res):
                """Per-feature [Win,2] sum/sumsq over node tiles via PE."""
                with tc.tile_pool(name="stp", bufs=1, space="PSUM") as stp:
                    ps_sum = stp.tile([Win, 1], F32, tag="pssum")
                    ps_sq = stp.tile([Win, 1], F32, tag="pssq")
                    for t in range(nt_count):
                        src = get_tile(t)
                        sq = st.tile([128, Win], F32, tag="sqb")
                        nc.scalar.activation(sq[:], src, AF.Square)
                        nc.tensor.matmul(ps_sum[:], src, onesc_sb[:],
                                         start=(t == 0), stop=(t == nt_count - 1))
                        nc.tensor.matmul(ps_sq[:], sq[:], onesc_sb[:],
                                         start=(t == 0), stop=(t == nt_count - 1))
                    nc.vector.tensor_copy(dst_res[:, 0:1], ps_sum[:])
                    nc.vector.tensor_copy(dst_res[:, 1:2], ps_sq[:])

            def stats_allreduce(get_tile, nt_count, Win, out_l, out_s):
                res = st.tile([128, 2], F32, tag="statres", name="statres")[:Win, :]
                stats_reduce(get_tile, nt_count, Win, res)
                if _abl("no_stats_ar"):
                    return res
                dmae.dma_start(out_l[:], res)
                nc.gpsimd.collective_compute(
                    "AllReduce", ALU.add, replica_groups=RG,
                    ins=[out_l[:]], outs=[out_s[:]])
                gst = st.tile([128, 2], F32, tag="statg", name="statg")[:Win, :]
                dmae.dma_start(gst, out_s[:])
                return gst

            def bn_fold(stats_sb, g_sb, b_sb, inv_count, W_sb, Win, Wout,
                        extra_bias=None):
                """stats [Win,2] -> W' = diag(s)@W and c = t@W (+extra)."""
                mean = st.tile([128, 1], F32, tag="bnm", name="bnm")[:Win, :]
                msq = st.tile([128, 1], F32, tag="bnq", name="bnq")[:Win, :]
                var = st.tile([128, 1], F32, tag="bnv", name="bnv")[:Win, :]
                sd = st.tile([128, 1], F32, tag="bnsd", name="bnsd")[:Win, :]
                s = st.tile([128, 1], F32, tag="bns", name="bns")[:Win, :]
                t = st.tile([128, 1], F32, tag="bnt", name="bnt")[:Win, :]
                Wp = st.tile([128, Wout], F32, tag="bnw", name="bnw")[:Win, :]
                c_sb = st.tile([1, Wout], F32, tag="bnc")
                nc.scalar.activation(mean, stats_sb[:, 0:1], AF.Copy,
                                     scale=float(inv_count))
                nc.scalar.activation(msq, stats_sb[:, 1:2], AF.Copy,
                                     scale=float(inv_count))
                nc.vector.tensor_mul(var, mean, mean)
                nc.vector.tensor_sub(var, msq, var)
                nc.scalar.activation(sd, var, AF.Sqrt, bias=eps_sb[:Win, :])
                nc.vector.reciprocal(s, sd)
                nc.vector.tensor_mul(s, s, g_sb)
                nc.vector.tensor_mul(t, mean, s)
                nc.vector.tensor_sub(t, b_sb, t)
                nc.vector.tensor_scalar_mul(Wp, W_sb, s)
                with tc.tile_pool(name="bnp", bufs=1, space="PSUM") as bnp:
                    c_ps = bnp.tile([1, Wout], F32, tag="bncp")
                    nc.tensor.matmul(c_ps[:], t, W_sb, start=True, stop=True)
                    if extra_bias is not None:
                        nc.vector.tensor_add(c_sb[:], c_ps[:], extra_bias)
                    else:
                        nc.vector.tensor_copy(c_sb[:], c_ps[:])
                return Wp, c_sb

            def forward():
                # ------------- layer 0: h0 = relu(BN(x) @ W_feat) -------------
                def x_tile(t):
                    xt = st.tile([128, F], F32, tag="xt")
                    if HWDGE:
                        xb = st.tile([128, F], BF16, tag="xtb")
                        dmae.dma_start(xb[:], x_d[t * 128:(t + 1) * 128, :])
                        nc.scalar.copy(xt[:], xb[:])
                    else:
                        dmae.dma_start(xt[:], x_d[t * 128:(t + 1) * 128, :])
                    return xt[:]

                gstF = stats_allreduce(x_tile, NT, F, statF_l, statF_s)
                WpF, cF = bn_fold(gstF, gfeat_sb, b2feat_sb, inv_n, Wf_sb, F, H)
                h_sb = per.tile([128, NT, H], F32, tag="h")
                with tc.tile_pool(name="l0p", bufs=2, space="PSUM") as l0p, \
                     tc.tile_pool(name="l0s", bufs=2) as l0s:
                    for t in range(NT):
                        xt = x_tile(t)
                        tp = l0p.tile([F, 128], F32, tag="l0T")
                        nc.tensor.transpose(tp[:], xt, ident_sb[:])
                        xT = l0s.tile([F, 128], F32, tag="l0hT")
                        nc.scalar.copy(xT[:], tp[:])
                        ps = l0p.tile([128, H], F32, tag="l0mm")
                        nc.tensor.matmul(ps[:], xT[:], WpF, start=True, stop=False)
                        nc.tensor.matmul(ps[:], onesr_sb[:], cF[:],
                                         start=False, stop=True)
                        nc.scalar.activation(h_sb[:, t, :], ps[:], AF.Relu)
                nc.vector.tensor_tensor(
                    h_sb[:], h_sb[:],
                    valid_sb[:].unsqueeze(2).broadcast_to([128, NT, H]), ALU.mult)

                # ------------- conv layers -------------
                hwpre_sb = per.tile([128, NT, H], F32, tag="hwpre")
                agg_sb = hwpre_sb  # accumulate in place once hwpre_d store has read it
                for li in range(3):
                    gstH = stats_allreduce(lambda t: h_sb[:, t, :], NT, H,
                                           statH_l[li], statH_s[li])
                    WpH, cH = bn_fold(gstH, gconv_sb[:, li:li + 1],
                                      b2conv_sb[:, li:li + 1], inv_n,
                                      Wc_sb[:, li * H:(li + 1) * H], H, H)
                    with tc.tile_pool(name="tfp", bufs=2, space="PSUM") as tfp, \
                         tc.tile_pool(name="tfs", bufs=2) as tfs:
                        for t in range(NT):
                            tp = tfp.tile([H, 128], F32, tag="tpT")
                            nc.tensor.transpose(tp[:], h_sb[:, t, :], ident_sb[:])
                            hT = tfs.tile([H, 128], F32, tag="hT")
                            nc.scalar.copy(hT[:], tp[:])
                            ps = tfp.tile([128, H], F32, tag="tpmm")
                            nc.tensor.matmul(ps[:], hT[:], WpH, start=True, stop=False)
                            nc.tensor.matmul(ps[:], onesr_sb[:], cH[:],
                                             start=False, stop=True)
                            nc.vector.tensor_scalar_mul(
                                hwpre_sb[:, t, :], ps[:], dinv_sb[:, t:t + 1])
                    dmae.dma_start(
                        hwpre_d[:].rearrange("(p t) e -> p t e", t=NT), hwpre_sb[:])
                    if not _abl("no_ag"):
                        nc.gpsimd.collective_compute(
                            "AllGather", ALU.bypass, replica_groups=RG,
                            ins=[hwpre_d[:]], outs=[hw_full[:]])

                    with tc.tile_pool(name="mpm", bufs=MSG_BUFS) as mpm, \
                         tc.tile_pool(name="mpo", bufs=2) as mpo, \
                         tc.tile_pool(name="mpp", bufs=3, space="PSUM") as mpp:
                      ps_cur = None
                      for _rep in range(0 if _abl("no_mp") else REPEAT_MP):
                        for ci, (ch, tile_off, ntl) in enumerate(calls):
                              nidx = ntl * 128
                              col0 = tile_off * 8
                              msg = mpm.tile([128, CALL_TILES, H], F32, tag="msg")
                              rows = min(CHUNK, NPAD - ch * CHUNK)
                              if _abl("no_gather"):
                                  nc.vector.memset(msg[:, :ntl, :], 0.0)
                              else:
                                  nc.gpsimd.dma_gather(
                                      out_ap=msg[:, :ntl, :],
                                      in_ap=hw_full[ch * CHUNK: ch * CHUNK + rows, :],
                                      idxs_ap=idx_sb[:, col0:col0 + ntl * 8],
                                      num_idxs=nidx, num_idxs_reg=nidx, elem_size=H,
                                      single_packet=SINGLE_PACKET,
                                      queue_num=ci % NQUEUES)
                              if _abl("gather_only"):
                                  continue
                              scol0, segs = call_segs[ci]
                              nseg = len(segs)
                              for b0 in range(0, nseg, OH_BATCH):
                                  nb = min(OH_BATCH, nseg - b0)
                                  sc0 = scol0 + b0
                                  S = (None if _abl("fixed_oh") else
                                       mpo.tile([128, OH_BATCH, 128], F32,
                                                tag="oneh"))
                                  if not _abl("fixed_oh"):
                                      nc.vector.tensor_tensor(
                                          S[:, :nb, :],
                                          slot_sb[:, sc0:sc0 + nb].unsqueeze(2)
                                          .broadcast_to([128, nb, 128]),
                                          iota_sb[:].unsqueeze(1)
                                          .broadcast_to([128, nb, 128]),
                                          ALU.is_equal)
                                  if _abl("no_mm"):
                                      continue
                                  for j in range(nb):
                                      tloc, first, last, blk = segs[b0 + j]
                                      if first:
                                          ps_cur = mpp.tile([128, H], F32,
                                                            tag="aggps")
                                      lhs = (ident_sb[:] if _abl("fixed_oh")
                                             else S[:, j, :])
                                      nc.tensor.matmul(
                                          ps_cur[:], lhs, msg[:, tloc, :],
                                          start=first, stop=last)
                                      if last:
                                          nc.vector.tensor_add(
                                              agg_sb[:, blk, :],
                                              agg_sb[:, blk, :], ps_cur[:])

                    nc.vector.tensor_tensor(
                        agg_sb[:], agg_sb[:],
                        dinv_sb[:].unsqueeze(2).broadcast_to([128, NT, H]),
                        ALU.mult)
                    nc.vector.tensor_tensor(
                        agg_sb[:], agg_sb[:],
                        bconv_sb[:, li, :].unsqueeze(1).broadcast_to([128, NT, H]),
                        ALU.add)
                    h_sb = per.tile([128, NT, H], F32, tag="h")
                    nc.scalar.activation(h_sb[:], agg_sb[:], AF.Relu)
                    nc.vector.tensor_tensor(
                        h_sb[:], h_sb[:],
                        valid_sb[:].unsqueeze(2).broadcast_to([128, NT, H]),
                        ALU.mult)

                # ------------- pooling -------------
                with tc.tile_pool(name="plp", bufs=1, space="PSUM") as plp, \
                     tc.tile_pool(name="pls", bufs=2) as pls:
                    ps_hg = plp.tile([128, H], F32, tag="pshg")
                    for t0 in range(0, NT, OH_BATCH):
                        nb = min(OH_BATCH, NT - t0)
                        S = pls.tile([128, OH_BATCH, 128], F32, tag="poneh")
                        nc.vector.tensor_tensor(
                            S[:, :nb, :],
                            batch_sb[:, t0:t0 + nb].unsqueeze(2)
                            .broadcast_to([128, nb, 128]),
                            iota_sb[:].unsqueeze(1).broadcast_to([128, nb, 128]),
                            ALU.is_equal)
                        for j in range(nb):
                            t = t0 + j
                            nc.tensor.matmul(ps_hg[:], S[:, j, :], h_sb[:, t, :],
                                             start=(t == 0), stop=(t == NT - 1))
                    hgp_sb = pls.tile([128, H], F32, tag="hgp")
                    nc.vector.tensor_copy(hgp_sb[:], ps_hg[:])
                    dmae.dma_start(hgp_d[:], hgp_sb[:])
                nc.gpsimd.collective_compute(
                    "AllReduce", ALU.add, replica_groups=RG,
                    ins=[hgp_d[:]], outs=[hg_sh[:]])

                # ------------- head (redundant on every core) -------------
                with tc.tile_pool(name="hd", bufs=1) as hd, \
                     tc.tile_pool(name="hdp", bufs=1, space="PSUM") as hdp:
                    hg = hd.tile([128, H], F32, tag="hg")
                    dmae.dma_start(hg[:], hg_sh[:])
                    stat2 = st.tile([128, 2], F32, tag="stat2", name="stat2")[:H, :]
                    stats_reduce(lambda t: hg[:], 1, H, stat2)
                    Wp2, c2 = bn_fold(stat2, gfc_sb, b2fc_sb, inv_g, Wfc_sb, H, H,
                                      extra_bias=bfc_sb[:])
                    tp = hdp.tile([H, 128], F32, tag="hdT")
                    nc.tensor.transpose(tp[:], hg[:, :H], ident_sb[:])
                    hgT = hd.tile([H, 128], F32, tag="hgT")
                    nc.scalar.copy(hgT[:], tp[:])
                    ps2 = hdp.tile([128, H], F32, tag="hdmm")
                    nc.tensor.matmul(ps2[:], hgT[:], Wp2, start=True, stop=False)
                    nc.tensor.matmul(ps2[:], onesr_sb[:], c2[:], start=False, stop=True)
                    hg2 = hd.tile([128, H], F32, tag="hg2")
                    nc.scalar.activation(hg2[:], ps2[:], AF.Relu)
                    nc.vector.tensor_scalar_mul(hg2[:], hg2[:], gvalid_sb[:])

                    stat3 = st.tile([128, 2], F32, tag="stat3", name="stat3")[:H, :]
                    stats_reduce(lambda t: hg2[:], 1, H, stat3)
                    Wp3, c3 = bn_fold(stat3, ghid_sb, b2hid_sb, inv_g, Wcls_sb, H, C,
                                      extra_bias=bcls_sb[:])
                    tp2 = hdp.tile([H, 128], F32, tag="hdT2")
                    nc.tensor.transpose(tp2[:], hg2[:, :H], ident_sb[:])
                    hg2T = hd.tile([H, 128], F32, tag="hg2T")
                    nc.scalar.copy(hg2T[:], tp2[:])
                    ps3 = hdp.tile([128, C], F32, tag="hdmm2")
                    nc.tensor.matmul(ps3[:], hg2T[:], Wp3, start=True, stop=False)
                    nc.tensor.matmul(ps3[:], onesr_sb[:], c3[:], start=False, stop=True)
                    out_sb = hd.tile([128, C], F32, tag="outsb")
                    nc.vector.tensor_copy(out_sb[:], ps3[:])
                    dmae.dma_start(out_d[:], out_sb[:])

            for _ in range(REPEAT_ALL):
                forward()

    nc.compile()
    return nc


def build_all(inputs):
    meta, in_maps = preprocess(inputs)
    nc = build_program(meta)
    return nc, meta, in_maps


def kernel(**inputs):
    from concourse import bass_utils
    nc, meta, in_maps = build_all(inputs)
    res = bass_utils.run_bass_kernel_spmd(
        nc, in_maps, core_ids=list(range(NCORES)))
    out = np.asarray(res.results[0]["out"], np.float32)
    return np.ascontiguousarray(out[:meta["G"], :])



# revision 4
# speedup vs baseline: 1.2148x; 1.2148x over previous
"""GCNNet forward on 8 Trainium2 NeuronCores (Bass/Tile).

Sharding: nodes in 8 contiguous blocks (SHARD rows each, tail zero-padded);
edges assigned to the core owning their *destination*. Per conv layer:

  hw_pre = dinv * (BN(h) @ W)   -- BN folded into the weights (W' = diag(s)W,
                                   c = t@W); per-feature stats via PE
                                   ones-matmul partition reduction + a tiny
                                   AllReduce of [feat, 2] sums
  AllGather hw_pre -> hw_full   -- gather source, tile-layout rows, bf16
                                   padded to 256B rows (gather elem minimum)
  per-edge messages come in via dma_gather (SWDGE custom ucode, int16
  indices, sources bucketed into 32768-row chunks)
  segment-sum on the PE: per 128-edge tile a one-hot matrix (DVE
  iota-compare against the dst slot, bf16) is matmul'd into a PSUM
  accumulator per (chunk, dst-block) group, then spilled into an f32 SBUF
  accumulator that was seeded with the self-loop term (hw_pre itself)
  h_next = relu(dinv * agg + b) * valid            (h kept in bf16)

The GCN normalization is separable (norm_e = dinv[src] * dinv[dst] for kept
edges), so no per-edge float arithmetic runs on any compute engine -- only
the gather DMA and the PE segment-sum. All matmul operands are bf16 (PSUM
accumulation stays fp32), which enables fast-weight-load and full-rate PE.
Pooling reuses the one-hot matmul over batch ids + an AllReduce; the tiny
2-layer head runs redundantly on every core in fp32.

All plain DMAs use nc.gpsimd (SWDGE): HWDGE (nc.sync) DMAs alongside the
custom SWDGE gather ucode crash the device (empirically bisected).
"""
import os
import sys

sys.path.insert(0, "/opt/trn_rl_repo")

import ml_dtypes
import numpy as np

import concourse.bacc as bacc
import concourse.mybir as mybir
import concourse.tile as tile

F32 = mybir.dt.float32
BF16 = mybir.dt.bfloat16
I16 = mybir.dt.int16

NCORES = 8
CHUNK = 32768          # gather-index range per int16 chunk
CALL_TILES = int(os.environ.get("CALL_TILES", "32"))  # tiles per dma_gather call
MSG_BUFS = int(os.environ.get("MSG_BUFS", "4"))
SINGLE_PACKET = os.environ.get("SINGLE_PACKET", "0") == "1"
HWDGE = os.environ.get("HWDGE", "0") == "1"  # plain DMAs on SP HWDGE
OH_BATCH = int(os.environ.get("OHB", "8"))  # tiles per DVE one-hot op
PAD_SLOT = 200         # one-hot slot for padding edges (matches nothing)
EPS = 1e-5
SCRATCH = 32768        # SWDGE descriptor carveout bytes/partition
REPEAT_MP = 1          # timing: repeat the message-passing phase per layer
REPEAT_ALL = 1         # timing: repeat the whole forward pass in one program
NQUEUES = int(os.environ.get("NQUEUES", "4"))  # SWDGE queues
ABLATE = ""            # timing: comma-list of no_gather|gather_only|no_ag|no_stats_ar|no_mp


def _abl(flag):
    return flag in ABLATE.split(",")


def _wrap_idx(a):
    """int16 indices -> SWDGE layout [128, n/16] (16-wrapped, 8x replicated)."""
    assert a.size % 16 == 0
    w = a.reshape(-1, 16).T.copy()
    return np.ascontiguousarray(np.tile(w, (8, 1)))


def _tab128(a, nt):
    """[nt*128] -> [128, nt] tile-column table (node l -> [l%128, l//128])."""
    return np.ascontiguousarray(a.reshape(nt, 128).T)


def preprocess(inputs):
    x = np.asarray(inputs["x"], np.float32)
    ei = np.asarray(inputs["edge_index"], np.int64)
    batch = np.asarray(inputs["batch"], np.int64)
    N, F = x.shape
    W_conv = np.asarray(inputs["W_conv"], np.float32)
    H = W_conv.shape[-1]
    W_cls = np.asarray(inputs["W_cls"], np.float32)
    C = W_cls.shape[-1]
    G = int(np.asarray(inputs["num_graphs"]))
    assert G <= 128 and F <= 128 and H <= 128

    SHARD = -(-N // (NCORES * 128)) * 128
    NT = SHARD // 128
    NPAD = NCORES * SHARD
    NCHUNK = -(-NPAD // CHUNK)

    row, col = ei[0], ei[1]
    keep = row != col
    row = row[keep]
    col = col[keep]

    deg = (np.bincount(row, minlength=N) + 1).astype(np.float32)
    dinv = (np.float32(1.0) / np.sqrt(deg)).astype(np.float32)
    dinv_pad = np.zeros(NPAD, np.float32)
    valid_pad = np.zeros(NPAD, np.float32)
    batch_pad = np.full(NPAD, PAD_SLOT, np.int16)
    dinv_pad[:N] = dinv
    valid_pad[:N] = 1.0
    batch_pad[:N] = batch.astype(np.int16)

    # hw_full rows use tile-layout: node l = t*128 + p on core k sits at
    # global row k*SHARD + p*NT + t.
    nglob = np.arange(NPAD, dtype=np.int64)
    n_local = nglob % SHARD
    tl_row = (nglob // SHARD) * SHARD + (n_local % 128) * NT + n_local // 128

    src_row = tl_row[row]
    dst_core = col // SHARD
    dst_local = col % SHARD

    NBLK = NT
    per_core = []
    cnts = np.zeros((NCORES, NCHUNK * NBLK), np.int64)
    for k in range(NCORES):
        m = dst_core == k
        r = src_row[m]
        c = dst_local[m]
        key = (r // CHUNK) * NBLK + (c >> 7)
        order = np.argsort(key, kind="stable")
        per_core.append((r[order], c[order], key[order]))
        cnts[k] = np.bincount(key, minlength=NCHUNK * NBLK)

    # Tightly-packed shared schedule: per (chunk, dst-block) group, slots =
    # max count across cores (no per-group 128-rounding); groups packed
    # back-to-back within each chunk, chunk tail padded to a tile boundary.
    # A 128-edge tile may span several groups; the one-hot matmul runs per
    # (tile, group-segment) on the partition subrange.
    gs_max = cnts.max(axis=0)

    calls = []           # (chunk, tile_off, n_tiles)
    seg_meta = []        # (gt, p0, p1, first, last, blk) per segment
    goff = np.zeros(NCHUNK * NBLK, np.int64)   # global slot offset per group
    n_tiles = 0
    for ch in range(NCHUNK):
        ch_slot0 = n_tiles * 128
        off = 0
        for blk in range(NBLK):
            g = ch * NBLK + blk
            goff[g] = ch_slot0 + off
            off += int(gs_max[g])
        S_ch = off
        nt_ch = -(-S_ch // 128)
        for blk in range(NBLK):
            g = ch * NBLK + blk
            s = int(gs_max[g])
            if s == 0:
                continue
            s0 = int(goff[g]) - ch_slot0
            s1 = s0 + s
            t0, t1 = s0 // 128, (s1 - 1) // 128
            for t in range(t0, t1 + 1):
                p0 = max(0, s0 - t * 128)
                p1 = min(128, s1 - t * 128)
                seg_meta.append(
                    (n_tiles + t, p0, p1, t == t0, t == t1, blk))
        off2 = 0
        while off2 < nt_ch:
            n = min(CALL_TILES, nt_ch - off2)
            calls.append((ch, n_tiles + off2, n))
            off2 += n
        n_tiles += nt_ch
    TOT = n_tiles * 128
    seg_meta.sort(key=lambda s: (s[0], s[1]))
    n_segs = len(seg_meta)

    # per-call segment lists: (seg_col0, [(tloc, first, last, blk), ...])
    call_segs = []
    si = 0
    for (ch, tile_off, ntl) in calls:
        s0 = si
        segs = []
        while si < n_segs and seg_meta[si][0] < tile_off + ntl:
            gt, p0, p1, first, last, blk = seg_meta[si]
            segs.append((gt - tile_off, first, last, blk))
            si += 1
        call_segs.append((s0, segs))
    assert si == n_segs

    src_tab = []
    slot_tab = []
    for k in range(NCORES):
        r, c, key = per_core[k]
        src_s = np.zeros(TOT, np.int64)          # pads gather row 0 of chunk
        slot_s = np.full(TOT, PAD_SLOT, np.int16)
        kcnt = cnts[k]
        start_of_group = np.concatenate([[0], np.cumsum(kcnt)[:-1]])
        within = np.arange(r.size, dtype=np.int64) - np.repeat(start_of_group, kcnt)
        pos = goff[key] + within
        src_s[pos] = r % CHUNK
        slot_s[pos] = (c & 127).astype(np.int16)
        src_tab.append(_wrap_idx(src_s.astype(np.int16)))
        seg_slot = np.full((n_segs, 128), PAD_SLOT, np.int16)
        for s, (gt, p0, p1, _f, _l, _b) in enumerate(seg_meta):
            seg_slot[s, p0:p1] = slot_s[gt * 128 + p0: gt * 128 + p1]
        slot_tab.append(np.ascontiguousarray(seg_slot.T))

    meta = dict(
        N=N, F=F, H=H, C=C, G=G, SHARD=SHARD, NT=NT, NPAD=NPAD,
        NCHUNK=NCHUNK, n_tiles=n_tiles, n_segs=n_segs,
        call_segs=call_segs, calls=calls,
    )

    params = dict(
        W_feat=np.ascontiguousarray(np.asarray(inputs["W_feat"], np.float32)),
        W_conv_cat=np.ascontiguousarray(
            W_conv.transpose(1, 0, 2).reshape(H, 3 * H)),
        W_fc=np.ascontiguousarray(np.asarray(inputs["W_fc"], np.float32)),
        W_cls=np.ascontiguousarray(W_cls),
        b_conv_rep=np.ascontiguousarray(np.broadcast_to(
            np.asarray(inputs["b_conv"], np.float32)[None, :, :], (128, 3, H))),
        g_conv=np.ascontiguousarray(np.asarray(inputs["bn_conv_g"], np.float32).T),
        b2_conv=np.ascontiguousarray(np.asarray(inputs["bn_conv_b"], np.float32).T),
        g_feat=np.asarray(inputs["bn_feat_g"], np.float32).reshape(F, 1).copy(),
        b2_feat=np.asarray(inputs["bn_feat_b"], np.float32).reshape(F, 1).copy(),
        g_fc=np.asarray(inputs["bn_fc_g"], np.float32).reshape(H, 1).copy(),
        b2_fc=np.asarray(inputs["bn_fc_b"], np.float32).reshape(H, 1).copy(),
        g_hid=np.asarray(inputs["bn_hidden_g"], np.float32).reshape(H, 1).copy(),
        b2_hid=np.asarray(inputs["bn_hidden_b"], np.float32).reshape(H, 1).copy(),
        b_fc=np.asarray(inputs["b_fc"], np.float32).reshape(1, H).copy(),
        b_cls=np.asarray(inputs["b_cls"], np.float32).reshape(1, C).copy(),
        identity=np.eye(128, dtype=np.float32),
        identity_bf=np.eye(128, dtype=ml_dtypes.bfloat16),
        iota=np.ascontiguousarray(
            np.broadcast_to(np.arange(128, dtype=np.int16)[None, :], (128, 128))),
        ones_col=np.ones((128, 1), np.float32),
        ones_col_bf=np.ones((128, 1), ml_dtypes.bfloat16),
        ones_row=np.ones((1, 128), np.float32),
        ones_row_bf=np.ones((1, 128), ml_dtypes.bfloat16),
        gvalid=(np.arange(128) < G).astype(np.float32).reshape(128, 1),
        eps_col=np.full((128, 1), EPS, np.float32),
    )

    x_pad = np.zeros((NPAD, F), np.float32)
    x_pad[:N] = x
    in_maps = []
    for k in range(NCORES):
        sl = slice(k * SHARD, (k + 1) * SHARD)
        m = dict(params)
        m["x"] = np.ascontiguousarray(x_pad[sl].astype(ml_dtypes.bfloat16))
        m["dinv_tab"] = _tab128(dinv_pad[sl], NT)
        m["valid_tab"] = _tab128(
            valid_pad[sl].astype(ml_dtypes.bfloat16), NT)
        m["batch_tab"] = np.ascontiguousarray(batch_pad[sl].reshape(NT, 128).T)
        m["src_idx"] = src_tab[k]
        m["slot_tab"] = slot_tab[k]
        in_maps.append(m)

    return meta, in_maps


def build_program(meta):
    N, F, H, C, G = meta["N"], meta["F"], meta["H"], meta["C"], meta["G"]
    SHARD, NT, NPAD = meta["SHARD"], meta["NT"], meta["NPAD"]
    n_tiles = meta["n_tiles"]
    call_segs = meta["call_segs"]
    calls = meta["calls"]
    n_segs = meta["n_segs"]
    TOT = n_tiles * 128

    nc = bacc.Bacc("TRN2", target_bir_lowering=False, debug=False,
                   num_devices=NCORES, dynamic_dma_scratch_size=SCRATCH,
                   num_swdge_queues=NQUEUES)

    x_d = nc.dram_tensor("x", [SHARD, F], BF16, kind="ExternalInput")
    src_idx_d = nc.dram_tensor("src_idx", [128, TOT // 16], I16, kind="ExternalInput")
    slot_tab_d = nc.dram_tensor("slot_tab", [128, n_segs], I16, kind="ExternalInput")
    batch_tab_d = nc.dram_tensor("batch_tab", [128, NT], I16, kind="ExternalInput")
    dinv_tab_d = nc.dram_tensor("dinv_tab", [128, NT], F32, kind="ExternalInput")
    valid_tab_d = nc.dram_tensor("valid_tab", [128, NT], BF16, kind="ExternalInput")
    Wf_d = nc.dram_tensor("W_feat", [F, H], F32, kind="ExternalInput")
    Wc_d = nc.dram_tensor("W_conv_cat", [H, 3 * H], F32, kind="ExternalInput")
    Wfc_d = nc.dram_tensor("W_fc", [H, H], F32, kind="ExternalInput")
    Wcls_d = nc.dram_tensor("W_cls", [H, C], F32, kind="ExternalInput")
    bconv_d = nc.dram_tensor("b_conv_rep", [128, 3, H], F32, kind="ExternalInput")
    gconv_d = nc.dram_tensor("g_conv", [H, 3], F32, kind="ExternalInput")
    b2conv_d = nc.dram_tensor("b2_conv", [H, 3], F32, kind="ExternalInput")
    gfeat_d = nc.dram_tensor("g_feat", [F, 1], F32, kind="ExternalInput")
    b2feat_d = nc.dram_tensor("b2_feat", [F, 1], F32, kind="ExternalInput")
    gfc_d = nc.dram_tensor("g_fc", [H, 1], F32, kind="ExternalInput")
    b2fc_d = nc.dram_tensor("b2_fc", [H, 1], F32, kind="ExternalInput")
    ghid_d = nc.dram_tensor("g_hid", [H, 1], F32, kind="ExternalInput")
    b2hid_d = nc.dram_tensor("b2_hid", [H, 1], F32, kind="ExternalInput")
    bfc_d = nc.dram_tensor("b_fc", [1, H], F32, kind="ExternalInput")
    bcls_d = nc.dram_tensor("b_cls", [1, C], F32, kind="ExternalInput")
    ident_d = nc.dram_tensor("identity", [128, 128], F32, kind="ExternalInput")
    identb_d = nc.dram_tensor("identity_bf", [128, 128], BF16, kind="ExternalInput")
    iota_d = nc.dram_tensor("iota", [128, 128], I16, kind="ExternalInput")
    onesc_d = nc.dram_tensor("ones_col", [128, 1], F32, kind="ExternalInput")
    onescb_d = nc.dram_tensor("ones_col_bf", [128, 1], BF16, kind="ExternalInput")
    onesr_d = nc.dram_tensor("ones_row", [1, 128], F32, kind="ExternalInput")
    onesrb_d = nc.dram_tensor("ones_row_bf", [1, 128], BF16, kind="ExternalInput")
    gvalid_d = nc.dram_tensor("gvalid", [128, 1], F32, kind="ExternalInput")
    eps_d = nc.dram_tensor("eps_col", [128, 1], F32, kind="ExternalInput")
    out_d = nc.dram_tensor("out", [128, C], F32, kind="ExternalOutput")

    # gather source rows are 128 bf16 = 256B (gather elem minimum); only the
    # first H columns hold data, the rest is padding the gather drags along.
    hwpre_d = nc.dram_tensor("hwpre_dram", [SHARD, 128], BF16, kind="Internal")
    hw_full = nc.dram_tensor("hw_full", [NPAD, 128], BF16, kind="Internal",
                             addr_space="Shared")
    statF_l = nc.dram_tensor("statF_l", [F, 2], F32, kind="Internal")
    statF_s = nc.dram_tensor("statF_s", [F, 2], F32, kind="Internal",
                             addr_space="Shared")
    statH_l = [nc.dram_tensor(f"statH_l{i}", [H, 2], F32, kind="Internal")
               for i in range(3)]
    statH_s = [nc.dram_tensor(f"statH_s{i}", [H, 2], F32, kind="Internal",
                              addr_space="Shared") for i in range(3)]
    hgp_d = nc.dram_tensor("hgp_dram", [128, H], F32, kind="Internal")
    hg_sh = nc.dram_tensor("hg_sh", [128, H], F32, kind="Internal",
                           addr_space="Shared")

    dmae = nc.sync if HWDGE else nc.gpsimd
    RG = [list(range(NCORES))]
    AF = mybir.ActivationFunctionType
    ALU = mybir.AluOpType
    inv_n = 1.0 / float(N)
    inv_g = 1.0 / float(G)

    with tile.TileContext(nc) as tc:
        with tc.tile_pool(name="per", bufs=1) as per, \
             tc.tile_pool(name="st", bufs=2) as st:
            slot_sb = per.tile([128, n_segs], I16, tag="slots")
            idx_sb = per.tile([128, n_tiles * 8], I16, tag="idxall")
            dinv_sb = per.tile([128, NT], F32, tag="dinv")
            valid_sb = per.tile([128, NT], BF16, tag="validt")
            batch_sb = per.tile([128, NT], I16, tag="batcht")
            iota_sb = per.tile([128, 128], I16, tag="iota")
            ident_sb = per.tile([128, 128], F32, tag="ident")
            identb_sb = per.tile([128, 128], BF16, tag="identb")
            onesc_sb = per.tile([128, 1], F32, tag="onesc")
            onescb_sb = per.tile([128, 1], BF16, tag="onescb")
            onesr_sb = per.tile([1, 128], F32, tag="onesr")
            onesrb_sb = per.tile([1, 128], BF16, tag="onesrb")
            gvalid_sb = per.tile([128, 1], F32, tag="gvalid")
            eps_sb = per.tile([128, 1], F32, tag="epsc")
            Wf_sb = per.tile([F, H], F32, tag="wf")
            Wc_sb = per.tile([H, 3 * H], F32, tag="wc")
            Wfc_sb = per.tile([H, H], F32, tag="wfc")
            Wcls_sb = per.tile([H, C], F32, tag="wcls")
            bconv_sb = per.tile([128, 3, H], F32, tag="bconv")
            gconv_sb = per.tile([H, 3], F32, tag="gconv")
            b2conv_sb = per.tile([H, 3], F32, tag="b2conv")
            gfeat_sb = per.tile([F, 1], F32, tag="gfeat")
            b2feat_sb = per.tile([F, 1], F32, tag="b2feat")
            gfc_sb = per.tile([H, 1], F32, tag="gfc")
            b2fc_sb = per.tile([H, 1], F32, tag="b2fc")
            ghid_sb = per.tile([H, 1], F32, tag="ghid")
            b2hid_sb = per.tile([H, 1], F32, tag="b2hid")
            bfc_sb = per.tile([1, H], F32, tag="bfc")
            bcls_sb = per.tile([1, C], F32, tag="bcls")

            for sb, d in [(slot_sb, slot_tab_d), (idx_sb, src_idx_d),
                          (dinv_sb, dinv_tab_d),
                          (valid_sb, valid_tab_d), (batch_sb, batch_tab_d),
                          (iota_sb, iota_d), (ident_sb, ident_d),
                          (identb_sb, identb_d),
                          (onesc_sb, onesc_d), (onescb_sb, onescb_d),
                          (onesr_sb, onesr_d), (onesrb_sb, onesrb_d),
                          (gvalid_sb, gvalid_d), (eps_sb, eps_d),
                          (Wf_sb, Wf_d), (Wc_sb, Wc_d), (Wfc_sb, Wfc_d),
                          (Wcls_sb, Wcls_d), (bconv_sb, bconv_d),
                          (gconv_sb, gconv_d), (b2conv_sb, b2conv_d),
                          (gfeat_sb, gfeat_d), (b2feat_sb, b2feat_d),
                          (gfc_sb, gfc_d), (b2fc_sb, b2fc_d),
                          (ghid_sb, ghid_d), (b2hid_sb, b2hid_d),
                          (bfc_sb, bfc_d), (bcls_sb, bcls_d)]:
                dmae.dma_start(sb[:], d[:])

            # ------------- helpers -------------
            def stats_reduce(get_tile, nt_count, Win, dst_res, ones, sq_dt):
                """Per-feature [Win,2] sum/sumsq over node tiles via PE."""
                with tc.tile_pool(name="stp", bufs=1, space="PSUM") as stp:
                    ps_sum = stp.tile([Win, 1], F32, tag="pssum")
                    ps_sq = stp.tile([Win, 1], F32, tag="pssq")
                    for t in range(nt_count):
                        src = get_tile(t)
                        sq = st.tile([128, Win], sq_dt, tag="sqb")
                        nc.scalar.activation(sq[:], src, AF.Square)
                        nc.tensor.matmul(ps_sum[:], src, ones,
                                         start=(t == 0), stop=(t == nt_count - 1))
                        nc.tensor.matmul(ps_sq[:], sq[:], ones,
                                         start=(t == 0), stop=(t == nt_count - 1))
                    nc.vector.tensor_copy(dst_res[:, 0:1], ps_sum[:])
                    nc.vector.tensor_copy(dst_res[:, 1:2], ps_sq[:])

            def stats_allreduce(get_tile, nt_count, Win, out_l, out_s):
                res = st.tile([128, 2], F32, tag="statres", name="statres")[:Win, :]
                stats_reduce(get_tile, nt_count, Win, res, onescb_sb[:], BF16)
                if _abl("no_stats_ar"):
                    return res
                dmae.dma_start(out_l[:], res)
                nc.gpsimd.collective_compute(
                    "AllReduce", ALU.add, replica_groups=RG,
                    ins=[out_l[:]], outs=[out_s[:]])
                gst = st.tile([128, 2], F32, tag="statg", name="statg")[:Win, :]
                dmae.dma_start(gst, out_s[:])
                return gst

            def bn_fold(stats_sb, g_sb, b_sb, inv_count, W_sb, Win, Wout,
                        extra_bias=None, out_dt=BF16):
                """stats [Win,2] -> W' = diag(s)@W and c = t@W (+extra)."""
                mean = st.tile([128, 1], F32, tag="bnm", name="bnm")[:Win, :]
                msq = st.tile([128, 1], F32, tag="bnq", name="bnq")[:Win, :]
                var = st.tile([128, 1], F32, tag="bnv", name="bnv")[:Win, :]
                sd = st.tile([128, 1], F32, tag="bnsd", name="bnsd")[:Win, :]
                s = st.tile([128, 1], F32, tag="bns", name="bns")[:Win, :]
                t = st.tile([128, 1], F32, tag="bnt", name="bnt")[:Win, :]
                Wp = st.tile([128, Wout], out_dt, tag="bnw", name="bnw")[:Win, :]
                c_sb = st.tile([1, Wout], out_dt, tag="bnc")
                nc.scalar.activation(mean, stats_sb[:, 0:1], AF.Copy,
                                     scale=float(inv_count))
                nc.scalar.activation(msq, stats_sb[:, 1:2], AF.Copy,
                                     scale=float(inv_count))
                nc.vector.tensor_mul(var, mean, mean)
                nc.vector.tensor_sub(var, msq, var)
                nc.scalar.activation(sd, var, AF.Sqrt, bias=eps_sb[:Win, :])
                nc.vector.reciprocal(s, sd)
                nc.vector.tensor_mul(s, s, g_sb)
                nc.vector.tensor_mul(t, mean, s)
                nc.vector.tensor_sub(t, b_sb, t)
                nc.vector.tensor_scalar_mul(Wp, W_sb, s)
                with tc.tile_pool(name="bnp", bufs=1, space="PSUM") as bnp:
                    c_ps = bnp.tile([1, Wout], F32, tag="bncp")
                    nc.tensor.matmul(c_ps[:], t, W_sb, start=True, stop=True)
                    if extra_bias is not None:
                        nc.vector.tensor_add(c_sb[:], c_ps[:], extra_bias)
                    else:
                        nc.vector.tensor_copy(c_sb[:], c_ps[:])
                return Wp, c_sb

            def forward():
                # ------------- layer 0: h0 = relu(BN(x) @ W_feat) -------------
                def x_tile(t):
                    xt = st.tile([128, F], BF16, tag="xt")
                    dmae.dma_start(xt[:], x_d[t * 128:(t + 1) * 128, :])
                    return xt[:]

                gstF = stats_allreduce(x_tile, NT, F, statF_l, statF_s)
                WpF, cF = bn_fold(gstF, gfeat_sb, b2feat_sb, inv_n, Wf_sb, F, H)
                h_sb = per.tile([128, NT, H], BF16, tag="h")
                with tc.tile_pool(name="l0p", bufs=2, space="PSUM") as l0p, \
                     tc.tile_pool(name="l0s", bufs=2) as l0s:
                    for t in range(NT):
                        xt = x_tile(t)
                        tp = l0p.tile([F, 128], BF16, tag="l0T")
                        nc.tensor.transpose(tp[:], xt, identb_sb[:])
                        xT = l0s.tile([F, 128], BF16, tag="l0hT")
                        nc.scalar.copy(xT[:], tp[:])
                        ps = l0p.tile([128, H], F32, tag="l0mm")
                        nc.tensor.matmul(ps[:], xT[:], WpF, start=True, stop=False)
                        nc.tensor.matmul(ps[:], onesrb_sb[:], cF[:],
                                         start=False, stop=True)
                        nc.scalar.activation(h_sb[:, t, :], ps[:], AF.Relu)
                nc.vector.tensor_tensor(
                    h_sb[:], h_sb[:],
                    valid_sb[:].unsqueeze(2).broadcast_to([128, NT, H]), ALU.mult)

                # ------------- conv layers -------------
                hwpre_sb = per.tile([128, NT, H], F32, tag="hwpre")
                hwb_sb = per.tile([128, NT, 128], BF16, tag="hwb")
                nc.vector.memset(hwb_sb[:], 0.0)
                agg_sb = hwpre_sb  # accumulate in place once hwb copy has read it
                for li in range(3):
                    gstH = stats_allreduce(lambda t: h_sb[:, t, :], NT, H,
                                           statH_l[li], statH_s[li])
                    WpH, cH = bn_fold(gstH, gconv_sb[:, li:li + 1],
                                      b2conv_sb[:, li:li + 1], inv_n,
                                      Wc_sb[:, li * H:(li + 1) * H], H, H)
                    with tc.tile_pool(name="tfp", bufs=2, space="PSUM") as tfp, \
                         tc.tile_pool(name="tfs", bufs=2) as tfs:
                        for t in range(NT):
                            tp = tfp.tile([H, 128], BF16, tag="tpT")
                            nc.tensor.transpose(tp[:], h_sb[:, t, :], identb_sb[:])
                            hT = tfs.tile([H, 128], BF16, tag="hT")
                            nc.scalar.copy(hT[:], tp[:])
                            ps = tfp.tile([128, H], F32, tag="tpmm")
                            nc.tensor.matmul(ps[:], hT[:], WpH, start=True, stop=False)
                            nc.tensor.matmul(ps[:], onesrb_sb[:], cH[:],
                                             start=False, stop=True)
                            nc.vector.tensor_scalar_mul(
                                hwpre_sb[:, t, :], ps[:], dinv_sb[:, t:t + 1])
                            nc.scalar.copy(hwb_sb[:, t, 0:H], hwpre_sb[:, t, :])
                    dmae.dma_start(
                        hwpre_d[:].rearrange("(p t) e -> p t e", t=NT), hwb_sb[:])
                    if not _abl("no_ag"):
                        nc.gpsimd.collective_compute(
                            "AllGather", ALU.bypass, replica_groups=RG,
                            ins=[hwpre_d[:]], outs=[hw_full[:]])

                    with tc.tile_pool(name="mpm", bufs=MSG_BUFS) as mpm, \
                         tc.tile_pool(name="mpo", bufs=2) as mpo, \
                         tc.tile_pool(name="mpp", bufs=3, space="PSUM") as mpp:
                      ps_cur = None
                      for _rep in range(0 if _abl("no_mp") else REPEAT_MP):
                        for ci, (ch, tile_off, ntl) in enumerate(calls):
                              nidx = ntl * 128
                              col0 = tile_off * 8
                              msg = mpm.tile([128, CALL_TILES, 128], BF16, tag="msg")
                              rows = min(CHUNK, NPAD - ch * CHUNK)
                              if _abl("no_gather"):
                                  nc.vector.memset(msg[:, :ntl, :], 0.0)
                              else:
                                  nc.gpsimd.dma_gather(
                                      out_ap=msg[:, :ntl, :],
                                      in_ap=hw_full[ch * CHUNK: ch * CHUNK + rows, :],
                                      idxs_ap=idx_sb[:, col0:col0 + ntl * 8],
                                      num_idxs=nidx, num_idxs_reg=nidx, elem_size=128,
                                      single_packet=SINGLE_PACKET,
                                      queue_num=ci % NQUEUES)
                              if _abl("gather_only"):
                                  continue
                              scol0, segs = call_segs[ci]
                              nseg = len(segs)
                              for b0 in range(0, nseg, OH_BATCH):
                                  nb = min(OH_BATCH, nseg - b0)
                                  sc0 = scol0 + b0
                                  S = (None if _abl("fixed_oh") else
                                       mpo.tile([128, OH_BATCH, 128], BF16,
                                                tag="oneh"))
                                  if not _abl("fixed_oh"):
                                      nc.vector.tensor_tensor(
                                          S[:, :nb, :],
                                          slot_sb[:, sc0:sc0 + nb].unsqueeze(2)
                                          .broadcast_to([128, nb, 128]),
                                          iota_sb[:].unsqueeze(1)
                                          .broadcast_to([128, nb, 128]),
                                          ALU.is_equal)
                                  if _abl("no_mm"):
                                      continue
                                  for j in range(nb):
                                      tloc, first, last, blk = segs[b0 + j]
                                      if first:
                                          ps_cur = mpp.tile([128, H], F32,
                                                            tag="aggps")
                                      lhs = (identb_sb[:] if _abl("fixed_oh")
                                             else S[:, j, :])
                                      nc.tensor.matmul(
                                          ps_cur[:], lhs, msg[:, tloc, 0:H],
                                          start=first, stop=last)
                                      if last:
                                          nc.vector.tensor_add(
                                              agg_sb[:, blk, :],
                                              agg_sb[:, blk, :], ps_cur[:])

                    nc.vector.tensor_tensor(
                        agg_sb[:], agg_sb[:],
                        dinv_sb[:].unsqueeze(2).broadcast_to([128, NT, H]),
                        ALU.mult)
                    nc.vector.tensor_tensor(
                        agg_sb[:], agg_sb[:],
                        bconv_sb[:, li, :].unsqueeze(1).broadcast_to([128, NT, H]),
                        ALU.add)
                    h_sb = per.tile([128, NT, H], BF16, tag="h")
                    nc.scalar.activation(h_sb[:], agg_sb[:], AF.Relu)
                    nc.vector.tensor_tensor(
                        h_sb[:], h_sb[:],
                        valid_sb[:].unsqueeze(2).broadcast_to([128, NT, H]),
                        ALU.mult)

                # ------------- pooling -------------
                with tc.tile_pool(name="plp", bufs=1, space="PSUM") as plp, \
                     tc.tile_pool(name="pls", bufs=2) as pls:
                    ps_hg = plp.tile([128, H], F32, tag="pshg")
                    for t0 in range(0, NT, OH_BATCH):
                        nb = min(OH_BATCH, NT - t0)
                        S = pls.tile([128, OH_BATCH, 128], BF16, tag="poneh")
                        nc.vector.tensor_tensor(
                            S[:, :nb, :],
                            batch_sb[:, t0:t0 + nb].unsqueeze(2)
                            .broadcast_to([128, nb, 128]),
                            iota_sb[:].unsqueeze(1).broadcast_to([128, nb, 128]),
                            ALU.is_equal)
                        for j in range(nb):
                            t = t0 + j
                            nc.tensor.matmul(ps_hg[:], S[:, j, :], h_sb[:, t, :],
                                             start=(t == 0), stop=(t == NT - 1))
                    hgp_sb = pls.tile([128, H], F32, tag="hgp")
                    nc.vector.tensor_copy(hgp_sb[:], ps_hg[:])
                    dmae.dma_start(hgp_d[:], hgp_sb[:])
                nc.gpsimd.collective_compute(
                    "AllReduce", ALU.add, replica_groups=RG,
                    ins=[hgp_d[:]], outs=[hg_sh[:]])

                # ------------- head (redundant on every core) -------------
                with tc.tile_pool(name="hd", bufs=1) as hd, \
                     tc.tile_pool(name="hdp", bufs=1, space="PSUM") as hdp:
                    hg = hd.tile([128, H], F32, tag="hg")
                    dmae.dma_start(hg[:], hg_sh[:])
                    stat2 = st.tile([128, 2], F32, tag="stat2", name="stat2")[:H, :]
                    stats_reduce(lambda t: hg[:], 1, H, stat2, onesc_sb[:], F32)
                    Wp2, c2 = bn_fold(stat2, gfc_sb, b2fc_sb, inv_g, Wfc_sb, H, H,
                                      extra_bias=bfc_sb[:], out_dt=F32)
                    tp = hdp.tile([H, 128], F32, tag="hdT")
                    nc.tensor.transpose(tp[:], hg[:, :H], ident_sb[:])
                    hgT = hd.tile([H, 128], F32, tag="hgT")
                    nc.scalar.copy(hgT[:], tp[:])
                    ps2 = hdp.tile([128, H], F32, tag="hdmm")
                    nc.tensor.matmul(ps2[:], hgT[:], Wp2, start=True, stop=False)
                    nc.tensor.matmul(ps2[:], onesr_sb[:], c2[:], start=False, stop=True)
                    hg2 = hd.tile([128, H], F32, tag="hg2")
                    nc.scalar.activation(hg2[:], ps2[:], AF.Relu)
                    nc.vector.tensor_scalar_mul(hg2[:], hg2[:], gvalid_sb[:])

                    stat3 = st.tile([128, 2], F32, tag="stat3", name="stat3")[:H, :]
                    stats_reduce(lambda t: hg2[:], 1, H, stat3, onesc_sb[:], F32)
                    Wp3, c3 = bn_fold(stat3, ghid_sb, b2hid_sb, inv_g, Wcls_sb, H, C,
                                      extra_bias=bcls_sb[:], out_dt=F32)
                    tp2 = hdp.tile([H, 128], F32, tag="hdT2")
                    nc.tensor.transpose(tp2[:], hg2[:, :H], ident_sb[:])
                    hg2T = hd.tile([H, 128], F32, tag="hg2T")
                    nc.scalar.copy(hg2T[:], tp2[:])
                    ps3 = hdp.tile([128, C], F32, tag="hdmm2")
                    nc.tensor.matmul(ps3[:], hg2T[:], Wp3, start=True, stop=False)
                    nc.tensor.matmul(ps3[:], onesr_sb[:], c3[:], start=False, stop=True)
                    out_sb = hd.tile([128, C], F32, tag="outsb")
                    nc.vector.tensor_copy(out_sb[:], ps3[:])
                    dmae.dma_start(out_d[:], out_sb[:])

            for _ in range(REPEAT_ALL):
                forward()

    nc.compile()
    return nc


def build_all(inputs):
    meta, in_maps = preprocess(inputs)
    nc = build_program(meta)
    return nc, meta, in_maps


def kernel(**inputs):
    from concourse import bass_utils
    nc, meta, in_maps = build_all(inputs)
    res = bass_utils.run_bass_kernel_spmd(
        nc, in_maps, core_ids=list(range(NCORES)))
    out = np.asarray(res.results[0]["out"], np.float32)
    return np.ascontiguousarray(out[:meta["G"], :])


# revision 18
# speedup vs baseline: 1.6243x; 1.3372x over previous
"""GCNNet forward on 8 Trainium2 NeuronCores (Bass/Tile).

Sharding: nodes in 8 contiguous blocks (SHARD rows each, tail zero-padded);
edges assigned to the core owning their *destination*. Per conv layer:

  hw_pre = dinv * (BN(h) @ W)   -- BN folded into the weights (W' = diag(s)W,
                                   c = t@W); per-feature stats via PE
                                   ones-matmul partition reduction + a tiny
                                   AllReduce of [feat, 2] sums
  AllGather hw_pre -> hw_full   -- gather source, tile-layout rows, bf16
                                   padded to 256B rows (gather elem minimum),
                                   double-buffered across layers
  per-edge messages come in via dma_gather (SWDGE custom ucode, int16
  indices, sources bucketed into 32768-row chunks, 4 SWDGE queues
  round-robin -- queue count parallelizes Q7 descriptor generation, the
  dominant cost at ~4ns/edge; a deep MSG_BUFS pipeline keeps transfers
  in flight under the serialized ring bookkeeping)
  segment-sum on the PE: per 128-edge tile a one-hot matrix (DVE
  iota-compare against the dst slot, bf16) is matmul'd into a PSUM
  accumulator per (chunk, dst-block) group, then spilled into an f32 SBUF
  accumulator that was seeded with the self-loop term (hw_pre itself)
  h_next = relu(dinv * agg + b) * valid            (h kept in bf16)

All matmul operands are bf16 (PSUM accumulation stays fp32), which enables
fast-weight-load and full-rate PE. Pooling reuses the one-hot matmul over
batch ids + an AllReduce; the tiny 2-layer head runs redundantly per core
in fp32. All plain DMAs use nc.gpsimd (SWDGE): HWDGE (nc.sync) DMAs
alongside the custom SWDGE gather ucode crash the device.
"""
import os
import sys

sys.path.insert(0, "/opt/trn_rl_repo")

import ml_dtypes
import numpy as np

import concourse.bacc as bacc
import concourse.mybir as mybir
import concourse.tile as tile

F32 = mybir.dt.float32
BF16 = mybir.dt.bfloat16
I16 = mybir.dt.int16

NCORES = 8
CHUNK = 32768          # gather-index range per int16 chunk
CALL_TILES = int(os.environ.get("CALL_TILES", "40"))  # tiles per dma_gather call
MSG_BUFS = int(os.environ.get("MSG_BUFS", "8"))
SINGLE_PACKET = False  # True crashes the device (empirical)
HWDGE = os.environ.get("HWDGE", "0") == "1"  # plain DMAs on SP HWDGE
OH_BATCH = int(os.environ.get("OHB", "8"))  # tiles per DVE one-hot op
# prepare_only + trigger_dma faults on this platform's firmware (verified
# with a minimal standalone repro); keep the pure-inline path.
PREP_CALLS = int(os.environ.get("PREPC", "0"))  # calls prepped ahead per layer
PAD_SLOT = 200         # one-hot slot for padding edges (matches nothing)
EPS = 1e-5
SCRATCH = 32768        # SWDGE descriptor carveout bytes/partition
REPEAT_ALL = 1         # timing: repeat the whole forward pass in one program
NQUEUES = int(os.environ.get("NQUEUES", "4"))  # SWDGE queues
ABLATE = ""            # timing: comma-list of no_gather|gather_only|no_ag|no_stats_ar|no_mp


def _abl(flag):
    return flag in ABLATE.split(",")


def _wrap_idx(a):
    """int16 indices -> SWDGE layout [128, n/16] (16-wrapped, 8x replicated)."""
    assert a.size % 16 == 0
    w = a.reshape(-1, 16).T.copy()
    return np.ascontiguousarray(np.tile(w, (8, 1)))


def _tab128(a, nt):
    """[nt*128] -> [128, nt] tile-column table (node l -> [l%128, l//128])."""
    return np.ascontiguousarray(a.reshape(nt, 128).T)


def preprocess(inputs):
    x = np.asarray(inputs["x"], np.float32)
    ei = np.asarray(inputs["edge_index"], np.int64)
    batch = np.asarray(inputs["batch"], np.int64)
    N, F = x.shape
    W_conv = np.asarray(inputs["W_conv"], np.float32)
    H = W_conv.shape[-1]
    W_cls = np.asarray(inputs["W_cls"], np.float32)
    C = W_cls.shape[-1]
    G = int(np.asarray(inputs["num_graphs"]))
    assert G <= 128 and F <= 128 and H <= 128

    SHARD = -(-N // (NCORES * 128)) * 128
    NT = SHARD // 128
    NPAD = NCORES * SHARD
    NCHUNK = -(-NPAD // CHUNK)

    row, col = ei[0], ei[1]
    keep = row != col
    row = row[keep]
    col = col[keep]

    deg = (np.bincount(row, minlength=N) + 1).astype(np.float32)
    dinv = (np.float32(1.0) / np.sqrt(deg)).astype(np.float32)
    dinv_pad = np.zeros(NPAD, np.float32)
    valid_pad = np.zeros(NPAD, np.float32)
    batch_pad = np.full(NPAD, PAD_SLOT, np.int16)
    dinv_pad[:N] = dinv
    valid_pad[:N] = 1.0
    batch_pad[:N] = batch.astype(np.int16)

    # hw_full rows use tile-layout: node l = t*128 + p on core k sits at
    # global row k*SHARD + p*NT + t.
    nglob = np.arange(NPAD, dtype=np.int64)
    n_local = nglob % SHARD
    tl_row = (nglob // SHARD) * SHARD + (n_local % 128) * NT + n_local // 128

    src_row = tl_row[row]
    dst_core = col // SHARD
    dst_local = col % SHARD

    NBLK = NT
    per_core = []
    cnts = np.zeros((NCORES, NCHUNK * NBLK), np.int64)
    for k in range(NCORES):
        m = dst_core == k
        r = src_row[m]
        c = dst_local[m]
        key = (r // CHUNK) * NBLK + (c >> 7)
        order = np.argsort(key, kind="stable")
        per_core.append((r[order], c[order], key[order]))
        cnts[k] = np.bincount(key, minlength=NCHUNK * NBLK)

    # Tightly-packed shared schedule: per (chunk, dst-block) group, slots =
    # max count across cores (no per-group 128-rounding); groups packed
    # back-to-back within each chunk, chunk tail padded to a tile boundary.
    # A 128-edge tile may span several groups; the one-hot matmul runs per
    # (tile, group-segment) on the partition subrange.
    gs_max = cnts.max(axis=0)

    calls = []           # (chunk, tile_off, n_tiles)
    seg_meta = []        # (gt, p0, p1, first, last, blk) per segment
    goff = np.zeros(NCHUNK * NBLK, np.int64)   # global slot offset per group
    n_tiles = 0
    for ch in range(NCHUNK):
        ch_slot0 = n_tiles * 128
        off = 0
        for blk in range(NBLK):
            g = ch * NBLK + blk
            goff[g] = ch_slot0 + off
            off += int(gs_max[g])
        S_ch = off
        nt_ch = -(-S_ch // 128)
        for blk in range(NBLK):
            g = ch * NBLK + blk
            s = int(gs_max[g])
            if s == 0:
                continue
            s0 = int(goff[g]) - ch_slot0
            s1 = s0 + s
            t0, t1 = s0 // 128, (s1 - 1) // 128
            for t in range(t0, t1 + 1):
                p0 = max(0, s0 - t * 128)
                p1 = min(128, s1 - t * 128)
                seg_meta.append(
                    (n_tiles + t, p0, p1, t == t0, t == t1, blk))
        off2 = 0
        while off2 < nt_ch:
            n = min(CALL_TILES, nt_ch - off2)
            calls.append((ch, n_tiles + off2, n))
            off2 += n
        n_tiles += nt_ch
    TOT = n_tiles * 128
    seg_meta.sort(key=lambda s: (s[0], s[1]))
    n_segs = len(seg_meta)

    # per-call segment lists: (seg_col0, [(tloc, first, last, blk), ...])
    call_segs = []
    si = 0
    for (ch, tile_off, ntl) in calls:
        s0 = si
        segs = []
        while si < n_segs and seg_meta[si][0] < tile_off + ntl:
            gt, p0, p1, first, last, blk = seg_meta[si]
            segs.append((gt - tile_off, first, last, blk))
            si += 1
        call_segs.append((s0, segs))
    assert si == n_segs

    src_tab = []
    slot_tab = []
    for k in range(NCORES):
        r, c, key = per_core[k]
        src_s = np.zeros(TOT, np.int64)          # pads gather row 0 of chunk
        slot_s = np.full(TOT, PAD_SLOT, np.int16)
        kcnt = cnts[k]
        start_of_group = np.concatenate([[0], np.cumsum(kcnt)[:-1]])
        within = np.arange(r.size, dtype=np.int64) - np.repeat(start_of_group, kcnt)
        pos = goff[key] + within
        src_s[pos] = r % CHUNK
        slot_s[pos] = (c & 127).astype(np.int16)
        src_tab.append(_wrap_idx(src_s.astype(np.int16)))
        seg_slot = np.full((n_segs, 128), PAD_SLOT, np.int16)
        for s, (gt, p0, p1, _f, _l, _b) in enumerate(seg_meta):
            seg_slot[s, p0:p1] = slot_s[gt * 128 + p0: gt * 128 + p1]
        slot_tab.append(np.ascontiguousarray(seg_slot.T))

    meta = dict(
        N=N, F=F, H=H, C=C, G=G, SHARD=SHARD, NT=NT, NPAD=NPAD,
        NCHUNK=NCHUNK, n_tiles=n_tiles, n_segs=n_segs,
        call_segs=call_segs, calls=calls,
    )

    params = dict(
        W_feat=np.ascontiguousarray(np.asarray(inputs["W_feat"], np.float32)),
        W_conv_cat=np.ascontiguousarray(
            W_conv.transpose(1, 0, 2).reshape(H, 3 * H)),
        W_fc=np.ascontiguousarray(np.asarray(inputs["W_fc"], np.float32)),
        W_cls=np.ascontiguousarray(W_cls),
        b_conv_rep=np.ascontiguousarray(np.broadcast_to(
            np.asarray(inputs["b_conv"], np.float32)[None, :, :], (128, 3, H))),
        g_conv=np.ascontiguousarray(np.asarray(inputs["bn_conv_g"], np.float32).T),
        b2_conv=np.ascontiguousarray(np.asarray(inputs["bn_conv_b"], np.float32).T),
        g_feat=np.asarray(inputs["bn_feat_g"], np.float32).reshape(F, 1).copy(),
        b2_feat=np.asarray(inputs["bn_feat_b"], np.float32).reshape(F, 1).copy(),
        g_fc=np.asarray(inputs["bn_fc_g"], np.float32).reshape(H, 1).copy(),
        b2_fc=np.asarray(inputs["bn_fc_b"], np.float32).reshape(H, 1).copy(),
        g_hid=np.asarray(inputs["bn_hidden_g"], np.float32).reshape(H, 1).copy(),
        b2_hid=np.asarray(inputs["bn_hidden_b"], np.float32).reshape(H, 1).copy(),
        b_fc=np.asarray(inputs["b_fc"], np.float32).reshape(1, H).copy(),
        b_cls=np.asarray(inputs["b_cls"], np.float32).reshape(1, C).copy(),
        identity=np.eye(128, dtype=np.float32),
        identity_bf=np.eye(128, dtype=ml_dtypes.bfloat16),
        iota=np.ascontiguousarray(
            np.broadcast_to(np.arange(128, dtype=np.int16)[None, :], (128, 128))),
        ones_col=np.ones((128, 1), np.float32),
        ones_col_bf=np.ones((128, 1), ml_dtypes.bfloat16),
        ones_row=np.ones((1, 128), np.float32),
        ones_row_bf=np.ones((1, 128), ml_dtypes.bfloat16),
        gvalid=(np.arange(128) < G).astype(np.float32).reshape(128, 1),
        eps_col=np.full((128, 1), EPS, np.float32),
    )

    x_pad = np.zeros((NPAD, F), np.float32)
    x_pad[:N] = x
    in_maps = []
    for k in range(NCORES):
        sl = slice(k * SHARD, (k + 1) * SHARD)
        m = dict(params)
        # x in tile-layout rows (row p*NT + t = local node t*128 + p) so one
        # dense DMA lands it as [128, NT, F] with 25KB contiguous/partition.
        xs = x_pad[sl].reshape(NT, 128, F).transpose(1, 0, 2).reshape(SHARD, F)
        m["x"] = np.ascontiguousarray(xs.astype(ml_dtypes.bfloat16))
        m["dinv_tab"] = _tab128(dinv_pad[sl], NT)
        m["valid_tab"] = _tab128(
            valid_pad[sl].astype(ml_dtypes.bfloat16), NT)
        m["batch_tab"] = np.ascontiguousarray(batch_pad[sl].reshape(NT, 128).T)
        m["src_idx"] = src_tab[k]
        m["slot_tab"] = slot_tab[k]
        in_maps.append(m)

    return meta, in_maps


def build_program(meta):
    N, F, H, C, G = meta["N"], meta["F"], meta["H"], meta["C"], meta["G"]
    SHARD, NT, NPAD = meta["SHARD"], meta["NT"], meta["NPAD"]
    n_tiles = meta["n_tiles"]
    call_segs = meta["call_segs"]
    calls = meta["calls"]
    n_segs = meta["n_segs"]
    TOT = n_tiles * 128
    n_calls = len(calls)

    nc = bacc.Bacc("TRN2", target_bir_lowering=False, debug=False,
                   num_devices=NCORES, dynamic_dma_scratch_size=SCRATCH,
                   num_swdge_queues=NQUEUES)

    x_d = nc.dram_tensor("x", [SHARD, F], BF16, kind="ExternalInput")
    src_idx_d = nc.dram_tensor("src_idx", [128, TOT // 16], I16, kind="ExternalInput")
    slot_tab_d = nc.dram_tensor("slot_tab", [128, n_segs], I16, kind="ExternalInput")
    batch_tab_d = nc.dram_tensor("batch_tab", [128, NT], I16, kind="ExternalInput")
    dinv_tab_d = nc.dram_tensor("dinv_tab", [128, NT], F32, kind="ExternalInput")
    valid_tab_d = nc.dram_tensor("valid_tab", [128, NT], BF16, kind="ExternalInput")
    Wf_d = nc.dram_tensor("W_feat", [F, H], F32, kind="ExternalInput")
    Wc_d = nc.dram_tensor("W_conv_cat", [H, 3 * H], F32, kind="ExternalInput")
    Wfc_d = nc.dram_tensor("W_fc", [H, H], F32, kind="ExternalInput")
    Wcls_d = nc.dram_tensor("W_cls", [H, C], F32, kind="ExternalInput")
    bconv_d = nc.dram_tensor("b_conv_rep", [128, 3, H], F32, kind="ExternalInput")
    gconv_d = nc.dram_tensor("g_conv", [H, 3], F32, kind="ExternalInput")
    b2conv_d = nc.dram_tensor("b2_conv", [H, 3], F32, kind="ExternalInput")
    gfeat_d = nc.dram_tensor("g_feat", [F, 1], F32, kind="ExternalInput")
    b2feat_d = nc.dram_tensor("b2_feat", [F, 1], F32, kind="ExternalInput")
    gfc_d = nc.dram_tensor("g_fc", [H, 1], F32, kind="ExternalInput")
    b2fc_d = nc.dram_tensor("b2_fc", [H, 1], F32, kind="ExternalInput")
    ghid_d = nc.dram_tensor("g_hid", [H, 1], F32, kind="ExternalInput")
    b2hid_d = nc.dram_tensor("b2_hid", [H, 1], F32, kind="ExternalInput")
    bfc_d = nc.dram_tensor("b_fc", [1, H], F32, kind="ExternalInput")
    bcls_d = nc.dram_tensor("b_cls", [1, C], F32, kind="ExternalInput")
    ident_d = nc.dram_tensor("identity", [128, 128], F32, kind="ExternalInput")
    identb_d = nc.dram_tensor("identity_bf", [128, 128], BF16, kind="ExternalInput")
    iota_d = nc.dram_tensor("iota", [128, 128], I16, kind="ExternalInput")
    onesc_d = nc.dram_tensor("ones_col", [128, 1], F32, kind="ExternalInput")
    onescb_d = nc.dram_tensor("ones_col_bf", [128, 1], BF16, kind="ExternalInput")
    onesr_d = nc.dram_tensor("ones_row", [1, 128], F32, kind="ExternalInput")
    onesrb_d = nc.dram_tensor("ones_row_bf", [1, 128], BF16, kind="ExternalInput")
    gvalid_d = nc.dram_tensor("gvalid", [128, 1], F32, kind="ExternalInput")
    eps_d = nc.dram_tensor("eps_col", [128, 1], F32, kind="ExternalInput")
    out_d = nc.dram_tensor("out", [128, C], F32, kind="ExternalOutput")

    # gather source rows are 128 bf16 = 256B (gather elem minimum); only the
    # first H columns hold hw_pre, the rest is whatever the staging tile held.
    hwpre_d = nc.dram_tensor("hwpre_dram", [SHARD, 128], BF16, kind="Internal")
    hw_full = [nc.dram_tensor(f"hw_full{i}", [NPAD, 128], BF16, kind="Internal",
                              addr_space="Shared") for i in range(2)]
    statF_l = nc.dram_tensor("statF_l", [F, 2], F32, kind="Internal")
    statF_s = nc.dram_tensor("statF_s", [F, 2], F32, kind="Internal",
                             addr_space="Shared")
    statH_l = [nc.dram_tensor(f"statH_l{i}", [H, 2], F32, kind="Internal")
               for i in range(3)]
    statH_s = [nc.dram_tensor(f"statH_s{i}", [H, 2], F32, kind="Internal",
                              addr_space="Shared") for i in range(3)]
    hgp_d = nc.dram_tensor("hgp_dram", [128, H], F32, kind="Internal")
    hg_sh = nc.dram_tensor("hg_sh", [128, H], F32, kind="Internal",
                           addr_space="Shared")

    dmae = nc.sync if HWDGE else nc.gpsimd
    RG = [list(range(NCORES))]
    AF = mybir.ActivationFunctionType
    ALU = mybir.AluOpType
    inv_n = 1.0 / float(N)
    inv_g = 1.0 / float(G)

    gsem = [nc.alloc_semaphore(f"gdma{q}") for q in range(NQUEUES)]
    psem = [nc.alloc_semaphore(f"gprep{q}") for q in range(NQUEUES)]
    prep_cnt = [0] * NQUEUES     # preps issued per queue (monotonic)
    mp_seq = [0]                 # global message-passing instance counter

    with tile.TileContext(nc) as tc:
        with tc.tile_pool(name="per", bufs=1) as per, \
             tc.tile_pool(name="st", bufs=2) as st, \
             tc.tile_pool(name="mpm", bufs=MSG_BUFS) as mpm, \
             tc.tile_pool(name="mpo", bufs=2) as mpo:
            slot_sb = per.tile([128, n_segs], I16, tag="slots")
            idx_sb = per.tile([128, n_tiles * 8], I16, tag="idxall")
            dinv_sb = per.tile([128, NT], F32, tag="dinv")
            valid_sb = per.tile([128, NT], BF16, tag="validt")
            batch_sb = per.tile([128, NT], I16, tag="batcht")
            iota_sb = per.tile([128, 128], I16, tag="iota")
            ident_sb = per.tile([128, 128], F32, tag="ident")
            identb_sb = per.tile([128, 128], BF16, tag="identb")
            onesc_sb = per.tile([128, 1], F32, tag="onesc")
            onescb_sb = per.tile([128, 1], BF16, tag="onescb")
            onesr_sb = per.tile([1, 128], F32, tag="onesr")
            onesrb_sb = per.tile([1, 128], BF16, tag="onesrb")
            gvalid_sb = per.tile([128, 1], F32, tag="gvalid")
            eps_sb = per.tile([128, 1], F32, tag="epsc")
            Wf_sb = per.tile([F, H], F32, tag="wf")
            Wc_sb = per.tile([H, 3 * H], F32, tag="wc")
            Wfc_sb = per.tile([H, H], F32, tag="wfc")
            Wcls_sb = per.tile([H, C], F32, tag="wcls")
            bconv_sb = per.tile([128, 3, H], F32, tag="bconv")
            gconv_sb = per.tile([H, 3], F32, tag="gconv")
            b2conv_sb = per.tile([H, 3], F32, tag="b2conv")
            gfeat_sb = per.tile([F, 1], F32, tag="gfeat")
            b2feat_sb = per.tile([F, 1], F32, tag="b2feat")
            gfc_sb = per.tile([H, 1], F32, tag="gfc")
            b2fc_sb = per.tile([H, 1], F32, tag="b2fc")
            ghid_sb = per.tile([H, 1], F32, tag="ghid")
            b2hid_sb = per.tile([H, 1], F32, tag="b2hid")
            bfc_sb = per.tile([1, H], F32, tag="bfc")
            bcls_sb = per.tile([1, C], F32, tag="bcls")

            for sb, d in [(slot_sb, slot_tab_d), (idx_sb, src_idx_d),
                          (dinv_sb, dinv_tab_d),
                          (valid_sb, valid_tab_d), (batch_sb, batch_tab_d),
                          (iota_sb, iota_d), (ident_sb, ident_d),
                          (identb_sb, identb_d),
                          (onesc_sb, onesc_d), (onescb_sb, onescb_d),
                          (onesr_sb, onesr_d), (onesrb_sb, onesrb_d),
                          (gvalid_sb, gvalid_d), (eps_sb, eps_d),
                          (Wf_sb, Wf_d), (Wc_sb, Wc_d), (Wfc_sb, Wfc_d),
                          (Wcls_sb, Wcls_d), (bconv_sb, bconv_d),
                          (gconv_sb, gconv_d), (b2conv_sb, b2conv_d),
                          (gfeat_sb, gfeat_d), (b2feat_sb, b2feat_d),
                          (gfc_sb, gfc_d), (b2fc_sb, b2fc_d),
                          (ghid_sb, ghid_d), (b2hid_sb, b2hid_d),
                          (bfc_sb, bfc_d), (bcls_sb, bcls_d)]:
                dmae.dma_start(sb[:], d[:])

            # ------------- helpers -------------
            def stats_reduce(get_tile, nt_count, Win, dst_res, ones, sq_dt):
                """Per-feature [Win,2] sum/sumsq over node tiles via PE."""
                with tc.tile_pool(name="stp", bufs=1, space="PSUM") as stp:
                    ps_sum = stp.tile([Win, 1], F32, tag="pssum")
                    ps_sq = stp.tile([Win, 1], F32, tag="pssq")
                    for t in range(nt_count):
                        src = get_tile(t)
                        sq = st.tile([128, Win], sq_dt, tag="sqb")
                        nc.scalar.activation(sq[:], src, AF.Square)
                        nc.tensor.matmul(ps_sum[:], src, ones,
                                         start=(t == 0), stop=(t == nt_count - 1))
                        nc.tensor.matmul(ps_sq[:], sq[:], ones,
                                         start=(t == 0), stop=(t == nt_count - 1))
                    nc.vector.tensor_copy(dst_res[:, 0:1], ps_sum[:])
                    nc.vector.tensor_copy(dst_res[:, 1:2], ps_sq[:])

            def stats_allreduce(get_tile, nt_count, Win, out_l, out_s):
                res = st.tile([128, 2], F32, tag="statres", name="statres")[:Win, :]
                stats_reduce(get_tile, nt_count, Win, res, onescb_sb[:], BF16)
                if _abl("no_stats_ar"):
                    return res
                dmae.dma_start(out_l[:], res)
                nc.gpsimd.collective_compute(
                    "AllReduce", ALU.add, replica_groups=RG,
                    ins=[out_l[:]], outs=[out_s[:]])
                gst = st.tile([128, 2], F32, tag="statg", name="statg")[:Win, :]
                dmae.dma_start(gst, out_s[:])
                return gst

            def bn_fold(stats_sb, g_sb, b_sb, inv_count, W_sb, Win, Wout,
                        extra_bias=None, out_dt=BF16):
                """stats [Win,2] -> W' = diag(s)@W and c = t@W (+extra)."""
                mean = st.tile([128, 1], F32, tag="bnm", name="bnm")[:Win, :]
                msq = st.tile([128, 1], F32, tag="bnq", name="bnq")[:Win, :]
                var = st.tile([128, 1], F32, tag="bnv", name="bnv")[:Win, :]
                sd = st.tile([128, 1], F32, tag="bnsd", name="bnsd")[:Win, :]
                s = st.tile([128, 1], F32, tag="bns", name="bns")[:Win, :]
                t = st.tile([128, 1], F32, tag="bnt", name="bnt")[:Win, :]
                Wp = st.tile([128, Wout], out_dt, tag="bnw", name="bnw")[:Win, :]
                c_sb = st.tile([1, Wout], out_dt, tag="bnc")
                nc.scalar.activation(mean, stats_sb[:, 0:1], AF.Copy,
                                     scale=float(inv_count))
                nc.scalar.activation(msq, stats_sb[:, 1:2], AF.Copy,
                                     scale=float(inv_count))
                nc.vector.tensor_mul(var, mean, mean)
                nc.vector.tensor_sub(var, msq, var)
                nc.scalar.activation(sd, var, AF.Sqrt, bias=eps_sb[:Win, :])
                nc.vector.reciprocal(s, sd)
                nc.vector.tensor_mul(s, s, g_sb)
                nc.vector.tensor_mul(t, mean, s)
                nc.vector.tensor_sub(t, b_sb, t)
                nc.vector.tensor_scalar_mul(Wp, W_sb, s)
                with tc.tile_pool(name="bnp", bufs=1, space="PSUM") as bnp:
                    c_ps = bnp.tile([1, Wout], F32, tag="bncp")
                    nc.tensor.matmul(c_ps[:], t, W_sb, start=True, stop=True)
                    if extra_bias is not None:
                        nc.vector.tensor_add(c_sb[:], c_ps[:], extra_bias)
                    else:
                        nc.vector.tensor_copy(c_sb[:], c_ps[:])
                return Wp, c_sb

            # ------------- message passing: prep / consume -------------
            mp_ctx = {}

            def issue_preps(hwf):
                """prepare_only desc-gen for the first PREP_CALLS calls.

                Descriptor generation reads only idx_sb (static), so these
                can be issued long before hwf's AllGather lands; the
                deferred data dep sits on the matching trigger_dma.
                Returns (msg tiles, per-queue psem targets)."""
                msgs = []
                for ci in range(min(PREP_CALLS, n_calls)):
                    ch, tile_off, ntl = calls[ci]
                    # queue 0 carries the plain SWDGE DMAs (hwpre store, AG
                    # staging) -- keep preps off it so the next AllGather
                    # isn't stuck behind prep desc-gen.
                    q = 1 + ci % (NQUEUES - 1) if NQUEUES > 1 else 0
                    nidx = ntl * 128
                    col0 = tile_off * 8
                    msg = mpm.tile([128, CALL_TILES, 128], BF16, tag="msg")
                    rows = min(CHUNK, NPAD - ch * CHUNK)
                    nc.gpsimd.dma_gather(
                        out_ap=msg[:, :ntl, :],
                        in_ap=hwf[ch * CHUNK: ch * CHUNK + rows, :],
                        idxs_ap=idx_sb[:, col0:col0 + ntl * 8],
                        num_idxs=nidx, num_idxs_reg=nidx, elem_size=128,
                        single_packet=SINGLE_PACKET, queue_num=q,
                        prepare_only=True, sem=gsem[q]).then_inc(psem[q], 1)
                    prep_cnt[q] += 1
                    msgs.append((msg, q))
                return msgs, list(prep_cnt)

            def consume_call(ci, msg):
                """one-hot segment-sum matmuls for one gather call."""
                scol0, segs = call_segs[ci]
                nseg = len(segs)
                for b0 in range(0, nseg, OH_BATCH):
                    nb = min(OH_BATCH, nseg - b0)
                    sc0 = scol0 + b0
                    S = (None if _abl("fixed_oh") else
                         mpo.tile([128, OH_BATCH, 128], BF16, tag="oneh"))
                    if not _abl("fixed_oh"):
                        nc.vector.tensor_tensor(
                            S[:, :nb, :],
                            slot_sb[:, sc0:sc0 + nb].unsqueeze(2)
                            .broadcast_to([128, nb, 128]),
                            iota_sb[:].unsqueeze(1)
                            .broadcast_to([128, nb, 128]),
                            ALU.is_equal)
                    if _abl("no_mm"):
                        continue
                    for j in range(nb):
                        tloc, first, last, blk = segs[b0 + j]
                        if first:
                            consume_call.ps = mp_ctx["mpp"].tile(
                                [128, H], F32, tag="aggps")
                        lhs = (identb_sb[:] if _abl("fixed_oh")
                               else S[:, j, :])
                        nc.tensor.matmul(
                            consume_call.ps[:], lhs, msg[:, tloc, 0:H],
                            start=first, stop=last)
                        if last:
                            agg = mp_ctx["agg"]
                            nc.vector.tensor_add(
                                agg[:, blk, :],
                                agg[:, blk, :], consume_call.ps[:])

            def run_mp(hwf, msgs, targets):
                """trigger prepped calls, then run the rest inline."""
                waited = set()
                mpp_cm = tc.tile_pool(name="mpp", bufs=3, space="PSUM")
                mp_ctx["mpp"] = mpp_cm.__enter__()
                for ci in range(n_calls):
                    ch, tile_off, ntl = calls[ci]
                    q = ci % NQUEUES
                    nidx = ntl * 128
                    col0 = tile_off * 8
                    rows = min(CHUNK, NPAD - ch * CHUNK)
                    if _abl("no_gather"):
                        msg = mpm.tile([128, CALL_TILES, 128], BF16, tag="msg")
                        nc.vector.memset(msg[:, :ntl, :], 0.0)
                    elif ci < len(msgs):
                        msg, q = msgs[ci]
                        if q not in waited:
                            nc.gpsimd.wait_ge(psem[q], targets[q])
                            waited.add(q)
                        nc.gpsimd.trigger_dma(count=1, queue_num=q)
                    else:
                        msg = mpm.tile([128, CALL_TILES, 128], BF16, tag="msg")
                        nc.gpsimd.dma_gather(
                            out_ap=msg[:, :ntl, :],
                            in_ap=hwf[ch * CHUNK: ch * CHUNK + rows, :],
                            idxs_ap=idx_sb[:, col0:col0 + ntl * 8],
                            num_idxs=nidx, num_idxs_reg=nidx, elem_size=128,
                            single_packet=SINGLE_PACKET, queue_num=q)
                    if _abl("gather_only"):
                        continue
                    consume_call(ci, msg)
                mpp_cm.__exit__(None, None, None)

            def forward():
                hwf0 = hw_full[mp_seq[0] % 2]
                if not (_abl("no_mp") or _abl("no_gather")):
                    msgs0, tgt0 = issue_preps(hwf0)
                else:
                    msgs0, tgt0 = [], list(prep_cnt)

                # ------------- layer 0: h0 = relu(BN(x) @ W_feat) -------------
                xfull = per.tile([128, NT, 128], BF16, tag="hwb")
                dmae.dma_start(
                    xfull[:, :, :F], x_d[:].rearrange("(p t) f -> p t f", t=NT))

                gstF = stats_allreduce(lambda t: xfull[:, t, :F], NT, F,
                                       statF_l, statF_s)
                WpF, cF = bn_fold(gstF, gfeat_sb, b2feat_sb, inv_n, Wf_sb, F, H)
                h_sb = per.tile([128, NT, H], BF16, tag="h")
                with tc.tile_pool(name="l0p", bufs=2, space="PSUM") as l0p, \
                     tc.tile_pool(name="l0s", bufs=2) as l0s:
                    for t in range(NT):
                        tp = l0p.tile([F, 128], BF16, tag="l0T")
                        nc.tensor.transpose(tp[:], xfull[:, t, :F], identb_sb[:])
                        xT = l0s.tile([F, 128], BF16, tag="l0hT")
                        nc.scalar.copy(xT[:], tp[:])
                        ps = l0p.tile([128, H], F32, tag="l0mm")
                        nc.tensor.matmul(ps[:], xT[:], WpF, start=True, stop=False)
                        nc.tensor.matmul(ps[:], onesrb_sb[:], cF[:],
                                         start=False, stop=True)
                        nc.scalar.activation(h_sb[:, t, :], ps[:], AF.Relu)
                nc.vector.tensor_tensor(
                    h_sb[:], h_sb[:],
                    valid_sb[:].unsqueeze(2).broadcast_to([128, NT, H]), ALU.mult)

                # ------------- conv layers -------------
                hwpre_sb = per.tile([128, NT, H], F32, tag="hwpre")
                agg = hwpre_sb  # accumulate in place once hwb copy has read it
                mp_ctx["agg"] = agg
                nonlocal_agg = {}
                for li in range(3):
                    hwf = hw_full[mp_seq[0] % 2]
                    mp_seq[0] += 1
                    msgs, tgts = (msgs0, tgt0) if li == 0 else (
                        nonlocal_agg.pop("msgs"), nonlocal_agg.pop("tgts"))

                    gstH = stats_allreduce(lambda t: h_sb[:, t, :], NT, H,
                                           statH_l[li], statH_s[li])
                    WpH, cH = bn_fold(gstH, gconv_sb[:, li:li + 1],
                                      b2conv_sb[:, li:li + 1], inv_n,
                                      Wc_sb[:, li * H:(li + 1) * H], H, H)
                    hwb_sb = per.tile([128, NT, 128], BF16, tag="hwb")
                    with tc.tile_pool(name="tfp", bufs=2, space="PSUM") as tfp, \
                         tc.tile_pool(name="tfs", bufs=2) as tfs:
                        for t in range(NT):
                            tp = tfp.tile([H, 128], BF16, tag="tpT")
                            nc.tensor.transpose(tp[:], h_sb[:, t, :], identb_sb[:])
                            hT = tfs.tile([H, 128], BF16, tag="hT")
                            nc.scalar.copy(hT[:], tp[:])
                            ps = tfp.tile([128, H], F32, tag="tpmm")
                            nc.tensor.matmul(ps[:], hT[:], WpH, start=True, stop=False)
                            nc.tensor.matmul(ps[:], onesrb_sb[:], cH[:],
                                             start=False, stop=True)
                            nc.vector.tensor_scalar_mul(
                                hwpre_sb[:, t, :], ps[:], dinv_sb[:, t:t + 1])
                            nc.scalar.copy(hwb_sb[:, t, 0:H], hwpre_sb[:, t, :])
                    dmae.dma_start(
                        hwpre_d[:].rearrange("(p t) e -> p t e", t=NT), hwb_sb[:])
                    if not _abl("no_ag"):
                        nc.gpsimd.collective_compute(
                            "AllGather", ALU.bypass, replica_groups=RG,
                            ins=[hwpre_d[:]], outs=[hwf[:]])

                    if not _abl("no_mp"):
                        run_mp(hwf, msgs, tgts)

                    # prep next layer's gathers while this layer's epilogue,
                    # stats, transform and AllGather run
                    if li < 2 and not (_abl("no_mp") or _abl("no_gather")):
                        nxt = issue_preps(hw_full[mp_seq[0] % 2])
                        nonlocal_agg["msgs"], nonlocal_agg["tgts"] = nxt
                    elif li < 2:
                        nonlocal_agg["msgs"], nonlocal_agg["tgts"] = \
                            [], list(prep_cnt)

                    nc.vector.tensor_tensor(
                        agg[:], agg[:],
                        dinv_sb[:].unsqueeze(2).broadcast_to([128, NT, H]),
                        ALU.mult)
                    nc.vector.tensor_tensor(
                        agg[:], agg[:],
                        bconv_sb[:, li, :].unsqueeze(1).broadcast_to([128, NT, H]),
                        ALU.add)
                    h_sb = per.tile([128, NT, H], BF16, tag="h")
                    nc.scalar.activation(h_sb[:], agg[:], AF.Relu)
                    nc.vector.tensor_tensor(
                        h_sb[:], h_sb[:],
                        valid_sb[:].unsqueeze(2).broadcast_to([128, NT, H]),
                        ALU.mult)

                # ------------- pooling -------------
                with tc.tile_pool(name="plp", bufs=1, space="PSUM") as plp, \
                     tc.tile_pool(name="pls", bufs=2) as pls:
                    ps_hg = plp.tile([128, H], F32, tag="pshg")
                    for t0 in range(0, NT, OH_BATCH):
                        nb = min(OH_BATCH, NT - t0)
                        S = pls.tile([128, OH_BATCH, 128], BF16, tag="poneh")
                        nc.vector.tensor_tensor(
                            S[:, :nb, :],
                            batch_sb[:, t0:t0 + nb].unsqueeze(2)
                            .broadcast_to([128, nb, 128]),
                            iota_sb[:].unsqueeze(1).broadcast_to([128, nb, 128]),
                            ALU.is_equal)
                        for j in range(nb):
                            t = t0 + j
                            nc.tensor.matmul(ps_hg[:], S[:, j, :], h_sb[:, t, :],
                                             start=(t == 0), stop=(t == NT - 1))
                    hgp_sb = pls.tile([128, H], F32, tag="hgp")
                    nc.vector.tensor_copy(hgp_sb[:], ps_hg[:])
                    dmae.dma_start(hgp_d[:], hgp_sb[:])
                nc.gpsimd.collective_compute(
                    "AllReduce", ALU.add, replica_groups=RG,
                    ins=[hgp_d[:]], outs=[hg_sh[:]])

                # ------------- head (redundant on every core) -------------
                with tc.tile_pool(name="hd", bufs=1) as hd, \
                     tc.tile_pool(name="hdp", bufs=1, space="PSUM") as hdp:
                    hg = hd.tile([128, H], F32, tag="hg")
                    dmae.dma_start(hg[:], hg_sh[:])
                    stat2 = st.tile([128, 2], F32, tag="stat2", name="stat2")[:H, :]
                    stats_reduce(lambda t: hg[:], 1, H, stat2, onesc_sb[:], F32)
                    Wp2, c2 = bn_fold(stat2, gfc_sb, b2fc_sb, inv_g, Wfc_sb, H, H,
                                      extra_bias=bfc_sb[:], out_dt=F32)
                    tp = hdp.tile([H, 128], F32, tag="hdT")
                    nc.tensor.transpose(tp[:], hg[:, :H], ident_sb[:])
                    hgT = hd.tile([H, 128], F32, tag="hgT")
                    nc.scalar.copy(hgT[:], tp[:])
                    ps2 = hdp.tile([128, H], F32, tag="hdmm")
                    nc.tensor.matmul(ps2[:], hgT[:], Wp2, start=True, stop=False)
                    nc.tensor.matmul(ps2[:], onesr_sb[:], c2[:], start=False, stop=True)
                    hg2 = hd.tile([128, H], F32, tag="hg2")
                    nc.scalar.activation(hg2[:], ps2[:], AF.Relu)
                    nc.vector.tensor_scalar_mul(hg2[:], hg2[:], gvalid_sb[:])

                    stat3 = st.tile([128, 2], F32, tag="stat3", name="stat3")[:H, :]
                    stats_reduce(lambda t: hg2[:], 1, H, stat3, onesc_sb[:], F32)
                    Wp3, c3 = bn_fold(stat3, ghid_sb, b2hid_sb, inv_g, Wcls_sb, H, C,
                                      extra_bias=bcls_sb[:], out_dt=F32)
                    tp2 = hdp.tile([H, 128], F32, tag="hdT2")
                    nc.tensor.transpose(tp2[:], hg2[:, :H], ident_sb[:])
                    hg2T = hd.tile([H, 128], F32, tag="hg2T")
                    nc.scalar.copy(hg2T[:], tp2[:])
                    ps3 = hdp.tile([128, C], F32, tag="hdmm2")
                    nc.tensor.matmul(ps3[:], hg2T[:], Wp3, start=True, stop=False)
                    nc.tensor.matmul(ps3[:], onesr_sb[:], c3[:], start=False, stop=True)
                    out_sb = hd.tile([128, C], F32, tag="outsb")
                    nc.vector.tensor_copy(out_sb[:], ps3[:])
                    dmae.dma_start(out_d[:], out_sb[:])

            for _ in range(REPEAT_ALL):
                forward()

    nc.compile()
    return nc


def build_all(inputs):
    meta, in_maps = preprocess(inputs)
    nc = build_program(meta)
    return nc, meta, in_maps


def kernel(**inputs):
    from concourse import bass_utils
    nc, meta, in_maps = build_all(inputs)
    res = bass_utils.run_bass_kernel_spmd(
        nc, in_maps, core_ids=list(range(NCORES)))
    out = np.asarray(res.results[0]["out"], np.float32)
    return np.ascontiguousarray(out[:meta["G"], :])
